# revision 2
# baseline (speedup 1.0000x reference)
"""Trainium2 Bass kernel (v8) for nn_ActorNetwork (GNN message passing actor).

Self-contained: hardcodes shapes B=32, K=64, D=4, DS=4, H=512, HH=256, NA=2.
Data-parallel over batch across 8 NeuronCores (4 samples/core).

Structure (per core, per 512-edge-token chunk):
- h1 = relu(U_i + V_j + b1): U=A@o, V=C@o+b1 once per core (PE); per chunk
  Pool broadcast-add (bf16) + ACT relu-cast to fp8.
- e2/e3: fp8 DoubleRow matmuls on PE (the bottleneck engine by design).
- h2/h3 PSUM evictions split ACT/DVE; j-reduction as one DVE instr per
  chunk into a bf16 agg_all tile.
- LN stats+normalize per sample, overlapped into the edge stream
  (normalize on Pool); node MLP + pools + heads in the tail with
  full-width instructions.
"""
import os as _os

import numpy as np

import concourse.bass as bass
import concourse.mybir as mybir
from concourse.bass_utils import run_bass_kernel_spmd
from concourse.tile import TileContext

# ---- problem constants ----
B, K, D, DS, H, HH, NA = 32, 64, 4, 4, 512, 256, 2
NCORES = 8
BSH = B // NCORES            # samples per core = 4
P = 128
FT = H // P                  # 4 feature tiles of hidden dim
TOK = BSH * K                # 256 node tokens per core
IBLK = 8                     # i-rows per edge chunk (8*64 = 512 tokens)
NCH = K // IBLK              # 8 chunks per sample
ET = K * K                   # 4096 edge tokens per sample

F32 = mybir.dt.float32
BF16 = mybir.dt.bfloat16
F8 = mybir.dt.float8e4
AF = mybir.ActivationFunctionType
AX = mybir.AxisListType
ALU = mybir.AluOpType
DR = mybir.MatmulPerfMode.DoubleRow

WARMUP_MM = int(_os.environ.get("K8_WARMUP_MM", "16"))
WARMUP2_MM = int(_os.environ.get("K8_WARMUP2_MM", "48"))
CHUNK_BUFS = int(_os.environ.get("K8_CHUNK_BUFS", "3"))
N_H2_ACT = int(_os.environ.get("K8_H2_ACT", "2"))   # h2 evicts on ACT (rest DVE)
N_H3_ACT = int(_os.environ.get("K8_H3_ACT", "2"))   # h3 evicts on ACT (rest DVE)
MM_FREE = int(_os.environ.get("K8_MM_FREE", "512"))  # moving cols per DR matmul
LN_INLINE = int(_os.environ.get("K8_LN_INLINE", "1"))  # per-sample LN in edge
THUNKS_PC = int(_os.environ.get("K8_THUNKS_PC", "2"))  # ln/node thunks per chunk
INC_STATS = int(_os.environ.get("K8_INC_STATS", "1"))  # incremental stats for last sample

EPS_S = (K * K) * 1e-5


def _split_excess_waits(nc, max_waits=1):
    """walrus in this container rejects >~2 sem waits on one instruction."""
    for f in nc.m.functions:
        for bb in f.blocks:
            insts = list(bb.instructions)
            new_list = []
            changed = False
            for inst in insts:
                si = inst.sync_info
                if si is not None and si.on_wait and len(si.on_wait) > max_waits:
                    waits = list(si.on_wait)
                    extra, keep = waits[:-max_waits], waits[-max_waits:]
                    for k0 in range(0, len(extra), max_waits):
                        chunk = extra[k0 : k0 + max_waits]
                        nop = mybir.InstNoOp(
                            name=f"{inst.name}-wsplit-{k0}",
                            engine=inst.engine,
                            ins=[],
                            outs=[],
                            sync_info=mybir.SyncInfo(on_wait=chunk, on_update=[]),
                        )
                        new_list.append(nop)
                        changed = True
                    si.on_wait = keep
                new_list.append(inst)
            if changed:
                bb.instructions = new_list


def build_bass():
    nc = bass.Bass("TRN2", debug=False, num_devices=NCORES)

    def dp(nm, sh, dt=F32):
        return nc.declare_dram_parameter(nm, sh, dt, isOutput=False)

    e1aT_d = dp("e1aT", [D, H], BF16)
    e1cT_d = dp("e1cT", [D, H], BF16)
    e2q_d = [dp(f"e2q{p}", [P, 2, H], F8) for p in range(2)]
    e3q_d = [dp(f"e3q{p}", [P, 2, H], F8) for p in range(2)]
    obs_d = dp("obsT", [D, TOK], BF16)
    st_d = dp("stateT", [DS, BSH], BF16)
    n1aT_d = dp("n1aT", [FT, P, H], BF16)
    n1oT_d = dp("n1oT", [D, H], BF16)
    n1sT_d = dp("n1sT", [DS, H], BF16)
    n2T_d = dp("n2T", [FT, P, HH], BF16)
    layerT_d = dp("layerT", [DS, H], BF16)
    mu1T_d = dp("mu1T", [2 * FT, P, 256], BF16)
    s1T_d = dp("s1T", [2 * FT, P, 256], BF16)
    mu2T_d = dp("mu2T", [2, P, 128], BF16)
    s2T_d = dp("s2T", [2, P, 128], BF16)
    mu3T_d = dp("mu3T", [P, NA], BF16)
    s3T_d = dp("s3T", [P, NA], BF16)
    bias_d = dp("bias_pack", [P, 32])
    mu_d = nc.declare_dram_parameter("mu", [NA, BSH], F32, isOutput=True)
    std_d = nc.declare_dram_parameter("std", [NA, BSH], F32, isOutput=True)

    with TileContext(nc) as tc:
        with (
            tc.tile_pool(name="w", bufs=1) as wp,
            tc.tile_pool(name="act", bufs=1) as pa,
            tc.tile_pool(name="chunk", bufs=CHUNK_BUFS) as cp,
            tc.tile_pool(name="ps", bufs=6, space="PSUM") as pp,
            tc.tile_pool(name="psln", bufs=2, space="PSUM") as ppl,
        ):
            def wload(nm, dram, idx=None, dt=F32):
                src = dram[:] if idx is None else dram[idx]
                t = wp.tile(list(src.shape), dt, name=nm, tag=nm)
                nc.sync.dma_start(out=t, in_=src)
                return t

            # ---- critical-path inputs first ----
            o_all = pa.tile([D, TOK], BF16, name="o_all", tag="o_all")
            nc.sync.dma_start(out=o_all, in_=obs_d[:])
            e1aT = wload("e1aT", e1aT_d, dt=BF16)
            e1cT = wload("e1cT", e1cT_d, dt=BF16)
            bias_t = wload("bias_t", bias_d)
            st_t = pa.tile([DS, BSH], BF16, name="st_t", tag="st_t")
            nc.sync.dma_start(out=st_t, in_=st_d[:])
            e2q = [wload(f"e2q{p}", e2q_d[p], dt=F8) for p in range(2)]
            e3q = [wload(f"e3q{p}", e3q_d[p], dt=F8) for p in range(2)]
            layerw = wload("layerw", layerT_d, dt=BF16)

            def bcol(i, rows=P):
                return bias_t[0:rows, i : i + 1]

            ones_col = pa.tile([P, 1], BF16, name="ones_col", tag="ones_col")
            nc.vector.memset(ones_col, 1.0)
            ones_row = pa.tile([1, P], F32, name="ones_row", tag="ones_row")
            nc.vector.memset(ones_row, 1.0)
            eps_t = pa.tile([1, 1], F32, name="eps_t", tag="eps_t")
            nc.vector.memset(eps_t, EPS_S)

            # trigger ACT table load early (overlaps DMA wait)
            dummy_a = pa.tile([1, 1], F32, name="dummy_a", tag="dummy_a")
            nc.scalar.activation(dummy_a, eps_t, AF.Relu)

            state_bc = pa.tile([DS, TOK], BF16, name="state_bc", tag="state_bc")
            nc.vector.tensor_copy(
                state_bc[:].rearrange("s (b k) -> s b k", b=BSH),
                st_t[:, :, None].broadcast_to([DS, BSH, K]),
            )

            # PE warmup while DMAs land (HAM un-throttle + clock ramp)
            wdu = pa.tile([P, 64], BF16, name="wdu", tag="wdu")
            nc.vector.memset(wdu, 0.0)
            psd = pp.tile([64, 64], F32, name="psd", tag="ps")
            for _w in range(WARMUP_MM):
                nc.tensor.matmul(psd, wdu, wdu, start=True, stop=True)

            # ---- U/V for e1-free h1: U = A@o, V = C@o + b1 ----
            U_sb = pa.tile([P, FT, TOK], BF16, name="U_sb", tag="U_sb")
            V_sb = pa.tile([P, FT, TOK], BF16, name="V_sb", tag="V_sb")
            for m in range(FT):
                msl = slice(m * P, (m + 1) * P)
                psu = pp.tile([P, TOK], F32, name=f"psu{m}", tag="ps")
                nc.tensor.matmul(psu, e1aT[:, msl], o_all, start=True,
                                 stop=True)
                nc.vector.tensor_copy(U_sb[:, m, :], psu)
                psv = pp.tile([P, TOK], F32, name=f"psv{m}", tag="ps")
                nc.tensor.matmul(psv, e1cT[:, msl], o_all, start=True,
                                 stop=True)
                nc.scalar.activation(V_sb[:, m, :], psv, AF.Identity,
                                     bias=bcol(0 + m))

            # st_feat early (no edge deps)
            xst = []
            for m in range(FT):
                msl = slice(m * P, (m + 1) * P)
                pst = pp.tile([P, BSH], F32, name=f"pst{m}", tag="ps")
                nc.tensor.matmul(pst, layerw[:, msl], st_t, start=True,
                                 stop=True)
                xm = pa.tile([P, BSH], BF16, name=f"xst{m}", tag=f"xst{m}")
                nc.scalar.activation(xm, pst, AF.Relu, bias=bcol(16 + m))
                xst.append(xm)


            psd2 = pp.tile([64, 64], F32, name="psd2", tag="ps")
            for _w in range(WARMUP2_MM):
                nc.tensor.matmul(psd2, wdu, wdu, start=True, stop=True)

            # agg/aggn accumulators [P, FT, TOK]
            agg_all = pa.tile([P, FT, TOK], BF16, name="agg_all", tag="agg_all")
            aggn_all = pa.tile([P, FT, TOK], BF16, name="aggn_all",
                               tag="aggn_all")

            # ---- per-sample LN stats + normalize ----
            def ln_stats(b, slab, cslice, first, last):
                # accumulate ones@agg and ones@agg^2 for agg cols cslice
                n = cslice.stop - cslice.start
                o0 = cslice.start - b * K
                sq = cp.tile([P, FT, n], BF16, name="sq_s", tag="sq_s")
                nc.scalar.activation(sq, agg_all[:, :, cslice], AF.Square)
                ps_sum = slab[0:1, o0 : o0 + n]
                ps_ssq = slab[0:1, K + o0 : K + o0 + n]
                for m in range(FT):
                    nc.tensor.matmul(ps_sum, ones_col, agg_all[:, m, cslice],
                                     start=(first and m == 0),
                                     stop=(last and m == FT - 1))
                for m in range(FT):
                    nc.tensor.matmul(ps_ssq, ones_col, sq[:, m, :],
                                     start=(first and m == 0),
                                     stop=(last and m == FT - 1))

            def sample_thunks(b, norm_eng=None, slab0=None):
                """LN + node MLP for sample b as a list of emission thunks.
                Each thunk's inputs are produced by earlier thunks/chunks so
                spreading them across chunk emissions avoids FIFO bubbles."""
                bsl = slice(b * K, (b + 1) * K)
                st = {}

                def t_stats():
                    if slab0 is not None:
                        st["slab"] = slab0
                    else:
                        st["slab"] = ppl.tile([P, 512], F32, name="ln_slab",
                                              tag="ln_slab")
                        ln_stats(b, st["slab"], bsl, True, True)

                def t_mean():
                    slab = st["slab"]
                    mean_r = cp.tile([1, K], F32, name="mean_r", tag="mean_r")
                    nc.vector.tensor_scalar_mul(mean_r, slab[0:1, 0:K],
                                                1.0 / H)
                    msq_r = cp.tile([1, K], F32, name="msq_r", tag="msq_r")
                    nc.vector.tensor_mul(msq_r, mean_r, mean_r)
                    var_r = cp.tile([1, K], F32, name="var_r", tag="var_r")
                    nc.vector.scalar_tensor_tensor(
                        var_r, slab[0:1, K : 2 * K], 1.0 / H, msq_r,
                        op0=ALU.mult, op1=ALU.subtract)
                    st["mean_r"], st["var_r"] = mean_r, var_r

                def t_rstd():
                    lnv_r = cp.tile([1, K], F32, name="lnv_r", tag="lnv_r")
                    nc.scalar.activation(lnv_r, st["var_r"], AF.Ln,
                                         bias=eps_t)
                    rstd_r = cp.tile([1, K], F32, name="rstd_r", tag="rstd_r")
                    nc.scalar.activation(rstd_r, lnv_r, AF.Exp, scale=-0.5)
                    st["rstd_r"] = rstd_r

                def t_bcast():
                    slab = st["slab"]
                    ps_mb = slab[:, 2 * K : 3 * K]
                    nc.tensor.matmul(ps_mb, ones_row, st["mean_r"],
                                     start=True, stop=True)
                    ps_rb = slab[:, 3 * K : 4 * K]
                    nc.tensor.matmul(ps_rb, ones_row, st["rstd_r"],
                                     start=True, stop=True)
                    mean_bc = cp.tile([P, K], F32, name="mean_bc",
                                      tag="mean_bc")
                    nc.scalar.copy(mean_bc, ps_mb)
                    rstd_bc = cp.tile([P, K], F32, name="rstd_bc",
                                      tag="rstd_bc")
                    nc.scalar.copy(rstd_bc, ps_rb)
                    st["mean_bc"], st["rstd_bc"] = mean_bc, rstd_bc

                def t_norm():
                    eng = norm_eng or nc.gpsimd
                    tmp = cp.tile([P, FT, K], BF16, name="ln_tmp",
                                  tag="ln_tmp")
                    eng.tensor_tensor(
                        tmp, agg_all[:, :, bsl],
                        st["mean_bc"][:, None, :].broadcast_to([P, FT, K]),
                        op=ALU.subtract)
                    eng.tensor_tensor(
                        aggn_all[:, :, bsl], tmp,
                        st["rstd_bc"][:, None, :].broadcast_to([P, FT, K]),
                        op=ALU.mult)

                def t_n1(m):
                    def f():
                        slab = st["slab"]
                        msl = slice(m * P, (m + 1) * P)
                        psn = slab[:, 256 + K * m : 256 + K * (m + 1)]
                        nc.tensor.matmul(psn, n1ow_t[0][:, msl],
                                         o_all[:, bsl],
                                         start=True, stop=False)
                        nc.tensor.matmul(psn, n1sw_t[0][:, msl],
                                         state_bc[:, bsl],
                                         start=False, stop=False)
                        for k2 in range(FT):
                            nc.tensor.matmul(psn, n1aw[k2][:, msl],
                                             aggn_all[:, k2, bsl],
                                             start=False,
                                             stop=(k2 == FT - 1))
                        nc.scalar.activation(hn1[m][:, bsl], psn, AF.Relu,
                                             bias=bcol(12 + m))
                    return f

                def t_n2(m2):
                    def f():
                        slab = st["slab"]
                        msl = slice(m2 * P, (m2 + 1) * P)
                        psn2 = slab[:, K * m2 : K * (m2 + 1)]
                        for k2 in range(FT):
                            nc.tensor.matmul(psn2, n2w[k2][:, msl],
                                             hn1[k2][:, bsl],
                                             start=(k2 == 0),
                                             stop=(k2 == FT - 1))
                        eng = nc.vector
                        with nc.allow_low_precision(reason="bf16 pool"):
                            eng.reduce_sum(
                                out=pool_sum[m2][:, b : b + 1],
                                in_=psn2[:, None, :], axis=AX.X)
                            eng.reduce_max(
                                out=pool_max[m2][:, b : b + 1],
                                in_=psn2[:, None, :], axis=AX.X)
                    return f

                return ([t_stats, t_mean, t_rstd, t_bcast, t_norm]
                        + [t_n1(m) for m in range(FT)]
                        + [t_n2(m2) for m2 in range(HH // P)])

            hn1 = []
            for m in range(FT):
                hn1.append(pa.tile([P, TOK], BF16, name=f"hn1_{m}",
                                   tag=f"hn1_{m}"))
            pool_sum, pool_max = [], []
            for m2 in range(HH // P):
                pool_sum.append(pa.tile([P, BSH], BF16, name=f"pool_s{m2}",
                                        tag=f"pool_s{m2}"))
                pool_max.append(pa.tile([P, BSH], BF16, name=f"pool_m{m2}",
                                        tag=f"pool_m{m2}"))

            # ---------------- edge MLP over K x K pairs ----------------
            chunk_list = [(b, ib) for b in range(BSH) for ib in range(NCH)]
            h1ts = {}
            t_pre = {}

            def emit_h1_add(ci):
                b, ib = chunk_list[ci]
                t = cp.tile([P, FT, IBLK * K], BF16, name="t_pre", tag="t_pre")
                bsl = slice(b * K, (b + 1) * K)
                usl = slice(b * K + ib * IBLK, b * K + (ib + 1) * IBLK)
                for q in range(2):
                    msl = slice(2 * q, 2 * q + 2)
                    nc.gpsimd.tensor_tensor(
                        t[:, msl, :].rearrange("p m (i j) -> p m i j", i=IBLK),
                        V_sb[:, msl, None, bsl].broadcast_to(
                            [P, 2, IBLK, K]),
                        U_sb[:, msl, usl, None].broadcast_to(
                            [P, 2, IBLK, K]),
                        op=ALU.add)
                t_pre[ci] = t

            def emit_h1_cast(ci):
                t = t_pre.pop(ci)
                h1t = cp.tile([P, FT, IBLK * K], F8, name="h1t", tag="h1t")
                for q in range(2):
                    msl = slice(2 * q, 2 * q + 2)
                    nc.scalar.activation(h1t[:, msl, :], t[:, msl, :], AF.Relu)
                h1ts[ci] = h1t

            NTH = (IBLK * K) // MM_FREE

            slabs = {}
            thunk_q = []
            n1aw, n1ow_t, n1sw_t, n2w = [], [], [], []
            emit_h1_add(0)
            emit_h1_cast(0)
            if len(chunk_list) > 1:
                emit_h1_add(1)
                emit_h1_cast(1)
            if len(chunk_list) > 2:
                emit_h1_add(2)
            pending_red = None
            for ci, (b, ib) in enumerate(chunk_list):
                c0 = b * K + ib * IBLK
                h1t = h1ts.pop(ci)

                # e2: fp8 DR, h2 -> fp8
                h2t = cp.tile([P, FT, IBLK * K], F8, name="h2t", tag="h2t")
                ps2 = []
                for m in range(FT):
                    msl = slice(m * P, (m + 1) * P)
                    p2 = pp.tile([P, IBLK * K], F32, name=f"ps2_{m}", tag="ps")
                    for th in range(NTH):
                        tsl = slice(th * MM_FREE, (th + 1) * MM_FREE)
                        for si in range(2):
                            nc.tensor.matmul(
                                p2[:, tsl], e2q[si][:, :, msl],
                                h1t[:, 2 * si : 2 * si + 2, tsl],
                                start=(si == 0), stop=(si == 1),
                                perf_mode=DR)
                    ps2.append(p2)
                for m in range(FT):
                    if m < N_H2_ACT:
                        nc.scalar.activation(h2t[:, m, :], ps2[m], AF.Relu,
                                             bias=bcol(4 + m))
                    else:
                        nc.vector.tensor_scalar(
                            h2t[:, m, :], ps2[m], bcol(4 + m), 0.0,
                            op0=ALU.add, op1=ALU.max)

                if ci + 3 < len(chunk_list):
                    emit_h1_add(ci + 3)
                if ci + 2 < len(chunk_list):
                    emit_h1_cast(ci + 2)

                # e3: fp8 DR; h3 evict + j-sum into agg_all
                h3t = cp.tile([P, FT, IBLK * K], BF16, name="h3t", tag="h3t")
                for m in range(FT):
                    msl = slice(m * P, (m + 1) * P)
                    p3 = pp.tile([P, IBLK * K], F32, name=f"ps3_{m}", tag="ps")
                    for th in range(NTH):
                        tsl = slice(th * MM_FREE, (th + 1) * MM_FREE)
                        for si in range(2):
                            nc.tensor.matmul(
                                p3[:, tsl], e3q[si][:, :, msl],
                                h2t[:, 2 * si : 2 * si + 2, tsl],
                                start=(si == 0), stop=(si == 1),
                                perf_mode=DR)
                    if m < N_H3_ACT:
                        nc.scalar.activation(h3t[:, m, :], p3, AF.Relu,
                                             bias=bcol(8 + m))
                    else:
                        nc.vector.tensor_scalar(
                            h3t[:, m, :], p3, bcol(8 + m), 0.0,
                            op0=ALU.add, op1=ALU.max)
                # deferred one-instr reduce for previous chunk
                if pending_red is not None:
                    pr_h3, pr_c0 = pending_red
                    with nc.allow_low_precision(reason="bf16 agg"):
                        nc.vector.reduce_sum(
                            out=agg_all[:, :, pr_c0 : pr_c0 + IBLK],
                            in_=pr_h3[:].rearrange("p m (i j) -> p m i j",
                                                   i=IBLK),
                            axis=AX.X)
                pending_red = (h3t, c0)
                if ci == 1:
                    # node weights: DMA streams during early chunks
                    for k in range(FT):
                        n1aw.append(wload(f"n1aw{k}", n1aT_d, k, BF16))
                    n1ow_t.append(wload("n1ow", n1oT_d, dt=BF16))
                    n1sw_t.append(wload("n1sw", n1sT_d, dt=BF16))
                    for k in range(FT):
                        n2w.append(wload(f"n2w{k}", n2T_d, k, BF16))
                if LN_INLINE and ci > 0 and ci % NCH == 0:
                    thunk_q.extend(sample_thunks(ci // NCH - 1))
                for _ in range(THUNKS_PC):
                    if thunk_q:
                        thunk_q.pop(0)()
                if LN_INLINE and INC_STATS and ci >= (BSH - 1) * NCH + 1:
                    # incremental stats for the last sample's landed agg cols
                    if ci == (BSH - 1) * NCH + 1:
                        last_slab = ppl.tile([P, 512], F32, name="ln_slab",
                                             tag="ln_slab")
                        slabs["last"] = last_slab
                    pc0 = (BSH - 1) * K + (ci - 1 - (BSH - 1) * NCH) * IBLK
                    ln_stats(BSH - 1, slabs["last"],
                             slice(pc0, pc0 + IBLK), first=(pc0 % K == 0),
                             last=False)

            pr_h3, pr_c0 = pending_red
            with nc.allow_low_precision(reason="bf16 agg"):
                for m in range(FT):
                    nc.vector.reduce_sum(
                        out=agg_all[:, m, pr_c0 : pr_c0 + IBLK],
                        in_=pr_h3[:, m, :].rearrange("p (i j) -> p i j",
                                                     i=IBLK),
                        axis=AX.X)
            while thunk_q:
                thunk_q.pop(0)()
            if LN_INLINE:
                if INC_STATS:
                    ln_stats(BSH - 1, slabs["last"],
                             slice(pr_c0, pr_c0 + IBLK), first=False,
                             last=True)
                    ths = sample_thunks(BSH - 1, norm_eng=nc.vector,
                                        slab0=slabs["last"])
                else:
                    ths = sample_thunks(BSH - 1, norm_eng=nc.vector)
                # node_pre before the LN scalar chain to hide its latency
                for t in ths:
                    t()
            else:
                for b in range(BSH):
                    for t in sample_thunks(b):
                        t()

            # ---- head weights (loads emitted late; DMA overlaps edge) ----
            mu1w = [wload(f"mu1w{k}", mu1T_d, k, BF16) for k in range(2 * FT)]
            s1w = [wload(f"s1w{k}", s1T_d, k, BF16) for k in range(2 * FT)]
            mu2w = [wload(f"mu2w{k}", mu2T_d, k, BF16) for k in range(2)]
            s2w = [wload(f"s2w{k}", s2T_d, k, BF16) for k in range(2)]
            mu3w = wload("mu3w", mu3T_d, dt=BF16)
            s3w = wload("s3w", s3T_d, dt=BF16)

            xs = xst + pool_sum + pool_max

            hl1 = {"mu": [], "s": []}
            for tag, w1, bc1 in (("mu", mu1w, 20), ("s", s1w, 24)):
                for m in range(2):
                    msl = slice(m * P, (m + 1) * P)
                    ph = pp.tile([P, BSH], F32, name=f"p{tag}1_{m}", tag="ps")
                    for k2 in range(2 * FT):
                        nc.tensor.matmul(ph, w1[k2][:, msl], xs[k2],
                                         start=(k2 == 0),
                                         stop=(k2 == 2 * FT - 1))
                    hm = pa.tile([P, BSH], BF16, name=f"h{tag}1_{m}",
                                 tag=f"h{tag}1_{m}")
                    eng = nc.scalar if tag == "mu" else None
                    if eng is not None:
                        eng.activation(hm, ph, AF.Relu, bias=bcol(bc1 + m))
                    else:
                        nc.vector.tensor_scalar(hm, ph, bcol(bc1 + m), 0.0,
                                                op0=ALU.add, op1=ALU.max)
                    hl1[tag].append(hm)
            hm2 = {}
            for tag, w2, bc2 in (("mu", mu2w, 22), ("s", s2w, 26)):
                ph2 = pp.tile([P, BSH], F32, name=f"p{tag}2", tag="ps")
                for k2 in range(2):
                    nc.tensor.matmul(ph2, w2[k2], hl1[tag][k2],
                                     start=(k2 == 0), stop=(k2 == 1))
                h2t_ = pa.tile([P, BSH], BF16, name=f"h{tag}2", tag=f"h{tag}2")
                if tag == "mu":
                    nc.scalar.activation(h2t_, ph2, AF.Relu, bias=bcol(bc2))
                else:
                    nc.vector.tensor_scalar(h2t_, ph2, bcol(bc2), 0.0,
                                            op0=ALU.add, op1=ALU.max)
                hm2[tag] = h2t_
            ph3_mu = pp.tile([NA, BSH], F32, name="pmu3", tag="ps")
            nc.tensor.matmul(ph3_mu, mu3w, hm2["mu"], start=True, stop=True)
            ph3_s = pp.tile([NA, BSH], F32, name="ps3h", tag="ps")
            nc.tensor.matmul(ph3_s, s3w, hm2["s"], start=True, stop=True)

            mu_sb = pa.tile([NA, BSH], F32, name="mu_sb", tag="mu_sb")
            nc.scalar.activation(mu_sb, ph3_mu, AF.Identity,
                                 bias=bcol(23, rows=NA))
            nc.sync.dma_start(out=mu_d[:], in_=mu_sb)
            std_sb = pa.tile([NA, BSH], F32, name="std_sb", tag="std_sb")
            nc.vector.tensor_scalar(std_sb, ph3_s, bcol(27, rows=NA), 0.0,
                                    op0=ALU.add, op1=ALU.bypass)
            nc.sync.dma_start(out=std_d[:], in_=std_sb)

    _split_excess_waits(nc)
    return nc


def _q8(x):
    import ml_dtypes
    return np.asarray(x, np.float32).astype(ml_dtypes.float8_e4m3)


def _f8f(x):
    return _q8(x).astype(np.float32)


def prep_weights(inp):
    """Host-side weight prep -> dict of replicated arrays."""
    import ml_dtypes
    bf = ml_dtypes.bfloat16

    def fb(a):
        return np.ascontiguousarray(np.asarray(a, np.float32), dtype=bf)

    e1_w = np.asarray(inp["e1_w"], np.float32)
    e2_w = np.asarray(inp["e2_w"], np.float32)
    e3_w = np.asarray(inp["e3_w"], np.float32)
    n1_w = np.asarray(inp["n1_w"], np.float32)
    ln_g = np.asarray(inp["ln_g"], np.float32)
    ln_b = np.asarray(inp["ln_b"], np.float32)
    n2_b = np.asarray(inp["n2_b"], np.float32)
    mu1_w = np.asarray(inp["mu1_w"], np.float32)
    s1_w = np.asarray(inp["s1_w"], np.float32)
    b1 = np.asarray(inp["e1_b"], np.float32)
    b2 = np.asarray(inp["e2_b"], np.float32)
    b3 = np.asarray(inp["e3_b"], np.float32)

    d = {}
    A_ = e1_w[:, :D]
    C_ = e1_w[:, D:]
    d["e1aT"] = fb(A_.T)
    d["e1cT"] = fb(C_.T)

    def pack_dr(wt):
        out = {}
        for p in range(2):
            arr = np.zeros((P, 2, H), np.float32)
            for q_ in range(2):
                ks = (2 * p + q_) * P
                arr[:, q_, :] = wt[ks : ks + P, :]
            out[p] = _q8(arr)
        return out

    w2t = e2_w.T
    q2 = _f8f(w2t)
    for p, a in pack_dr(q2).items():
        d[f"e2q{p}"] = a
    e2_eff = q2.T

    w3t = e3_w.T
    q3 = _f8f(w3t)
    for p, a in pack_dr(q3).items():
        d[f"e3q{p}"] = a
    e3_eff = q3.T

    # ---- bias corrections via subsampled calibration means ----
    obs = np.asarray(inp["obs"], np.float32)
    o = obs.transpose(0, 2, 1).reshape(B * K, D)
    obf = np.asarray(np.asarray(o, np.float32).astype(bf), np.float32)
    Abf = np.asarray(fb(A_), np.float32)
    Cbf = np.asarray(fb(C_), np.float32)
    U = (obf @ Abf.T).reshape(B, K, H)
    V = (obf @ Cbf.T).reshape(B, K, H)
    jsub = np.arange(0, K, 8)
    m1 = np.zeros(H, np.float64)
    m2 = np.zeros(H, np.float64)
    for bi in range(B):
        h1 = np.maximum(U[bi][:, None, :] + V[bi][None, jsub, :] + b1, 0.0)
        h1 = _f8f(h1.reshape(-1, H))
        m1 += h1.mean(0) / B
        h2 = np.maximum(h1 @ e2_w.T + b2, 0.0)
        m2 += h2.mean(0) / B
    m1 = m1.astype(np.float32)
    m2 = m2.astype(np.float32)
    b2_eff = b2 - (e2_eff - e2_w) @ m1
    b3_eff = b3 - (e3_eff - e3_w) @ m2

    d["n1aT"] = fb((n1_w[:, D : D + H] * ln_g[None, :]).T.reshape(FT, P, H))
    d["n1oT"] = fb(n1_w[:, :D].T)
    d["n1sT"] = fb(n1_w[:, D + H :].T)
    d["n2T"] = fb(np.asarray(inp["n2_w"], np.float32).T.reshape(FT, P, HH))
    d["layerT"] = fb(np.asarray(inp["layer_w"], np.float32).T)

    mu1 = mu1_w.copy()
    mu1[:, H : H + HH] *= 1.0 / K
    d["mu1T"] = fb(mu1.T.reshape(2 * FT, P, 256))
    s1 = s1_w.copy()
    s1[:, H : H + HH] *= 1.0 / K
    d["s1T"] = fb(s1.T.reshape(2 * FT, P, 256))
    d["mu2T"] = fb(np.asarray(inp["mu2_w"], np.float32).T.reshape(2, P, 128))
    d["s2T"] = fb(np.asarray(inp["s2_w"], np.float32).T.reshape(2, P, 128))
    d["mu3T"] = fb(np.asarray(inp["mu3_w"], np.float32).T)
    d["s3T"] = fb(np.asarray(inp["s3_w"], np.float32).T)

    n1_b_eff = np.asarray(inp["n1_b"], np.float32) + n1_w[:, D : D + H] @ ln_b
    mu1_b_eff = (np.asarray(inp["mu1_b"], np.float32)
                 + (mu1_w[:, H : H + HH] + mu1_w[:, H + HH :]) @ n2_b)
    s1_b_eff = (np.asarray(inp["s1_b"], np.float32)
                + (s1_w[:, H : H + HH] + s1_w[:, H + HH :]) @ n2_b)

    bp = np.zeros((P, 32), np.float32)
    bp[:, 0:4] = b1.reshape(FT, P).T
    bp[:, 4:8] = b2_eff.reshape(FT, P).T
    bp[:, 8:12] = b3_eff.reshape(FT, P).T
    bp[:, 12:16] = n1_b_eff.reshape(FT, P).T
    bp[:, 16:20] = np.asarray(inp["layer_b"], np.float32).reshape(FT, P).T
    bp[:, 20:22] = mu1_b_eff.reshape(2, P).T
    bp[:, 22] = np.asarray(inp["mu2_b"], np.float32)
    bp[0:NA, 23] = np.asarray(inp["mu3_b"], np.float32)
    bp[:, 24:26] = s1_b_eff.reshape(2, P).T
    bp[:, 26] = np.asarray(inp["s2_b"], np.float32)
    bp[0:NA, 27] = np.asarray(inp["s3_b"], np.float32)
    d["bias_pack"] = bp
    return d


def make_in_maps(inputs):
    import ml_dtypes
    bf = ml_dtypes.bfloat16
    w = prep_weights(inputs)
    obs = np.asarray(inputs["obs"], np.float32)
    state = np.asarray(inputs["state"], np.float32)
    in_maps = []
    for c in range(NCORES):
        m = dict(w)
        ob = obs[c * BSH : (c + 1) * BSH]          # [BSH, D, K]
        m["obsT"] = np.ascontiguousarray(
            ob.transpose(1, 0, 2).reshape(D, TOK), dtype=bf)
        m["stateT"] = np.ascontiguousarray(
            state[c * BSH : (c + 1) * BSH].T, dtype=bf)
        in_maps.append(m)
    return in_maps


_NC_CACHE = {}


def get_nc():
    key = (WARMUP_MM, WARMUP2_MM, CHUNK_BUFS, N_H2_ACT, N_H3_ACT, MM_FREE,
           LN_INLINE, THUNKS_PC, INC_STATS)
    if key not in _NC_CACHE:
        _NC_CACHE[key] = build_bass()
    return _NC_CACHE[key]


def run(in_maps, trace=False, **kw):
    nc = get_nc()
    return run_bass_kernel_spmd(nc, in_maps, core_ids=list(range(NCORES)),
                                trace=trace, **kw)


def gather(res_list):
    mu = np.concatenate([r["mu"].T for r in res_list], axis=0)
    pre = np.concatenate([r["std"].T for r in res_list],
                         axis=0).astype(np.float64)
    std = np.clip(np.log1p(np.exp(pre)) + 0.001, 0.1, 2.0)
    return mu.astype(np.float32), std.astype(np.float32)


def kernel(**inputs):
    res = run(make_in_maps(inputs))
    return gather(res.results)


# revision 3
# speedup vs baseline: 1.0084x; 1.0084x over previous
"""Trainium2 Bass kernel (v13) for nn_ActorNetwork (GNN message passing actor).

Self-contained: hardcodes shapes B=32, K=64, D=4, DS=4, H=512, HH=256, NA=2.
Data-parallel over batch across 8 NeuronCores (4 samples/core).

Structure (per core, per 512-edge-token chunk):
- h1 = relu(U_i + V_j + b1): U=A@o, V=C@o+b1 once per core (PE); per chunk
  Pool broadcast-add (bf16) + ACT relu-cast to fp8.
- e2/e3: fp8 DoubleRow matmuls on PE (the bottleneck engine by design).
- h2/h3 PSUM evictions split ACT/DVE; j-reduction as one DVE instr per
  chunk into a bf16 agg_all tile.
- LN stats+normalize per sample, overlapped into the edge stream
  (normalize on Pool); node MLP + pools + heads in the tail with
  full-width instructions.
"""
import os as _os

import numpy as np

import concourse.bass as bass
import concourse.mybir as mybir
from concourse.bass_utils import run_bass_kernel_spmd
from concourse.tile import TileContext

# ---- problem constants ----
B, K, D, DS, H, HH, NA = 32, 64, 4, 4, 512, 256, 2
NCORES = 8
BSH = B // NCORES            # samples per core = 4
P = 128
FT = H // P                  # 4 feature tiles of hidden dim
TOK = BSH * K                # 256 node tokens per core
IBLK = 8                     # i-rows per edge chunk (8*64 = 512 tokens)
NCH = K // IBLK              # 8 chunks per sample
ET = K * K                   # 4096 edge tokens per sample

F32 = mybir.dt.float32
BF16 = mybir.dt.bfloat16
F8 = mybir.dt.float8e4
AF = mybir.ActivationFunctionType
AX = mybir.AxisListType
ALU = mybir.AluOpType
DR = mybir.MatmulPerfMode.DoubleRow

WARMUP_MM = int(_os.environ.get("K13_WARMUP_MM", "16"))
WARMUP2_MM = int(_os.environ.get("K13_WARMUP2_MM", "48"))
CHUNK_BUFS = int(_os.environ.get("K13_CHUNK_BUFS", "3"))
N_H2_ACT = int(_os.environ.get("K13_H2_ACT", "2"))   # h2 evicts on ACT (rest DVE)
N_H3_ACT = int(_os.environ.get("K13_H3_ACT", "2"))   # h3 evicts on ACT (rest DVE)
MM_FREE = int(_os.environ.get("K13_MM_FREE", "512"))  # moving cols per DR matmul
LN_INLINE = int(_os.environ.get("K13_LN_INLINE", "1"))  # per-sample LN in edge
THUNKS_PC = int(_os.environ.get("K13_THUNKS_PC", "2"))  # ln/node thunks per chunk
INC_STATS = int(_os.environ.get("K13_INC_STATS", "1"))  # incremental stats for last sample

EPS_S = (K * K) * 1e-5


def _split_excess_waits(nc, max_waits=1):
    """walrus in this container rejects >~2 sem waits on one instruction."""
    for f in nc.m.functions:
        for bb in f.blocks:
            insts = list(bb.instructions)
            new_list = []
            changed = False
            for inst in insts:
                si = inst.sync_info
                if si is not None and si.on_wait and len(si.on_wait) > max_waits:
                    waits = list(si.on_wait)
                    extra, keep = waits[:-max_waits], waits[-max_waits:]
                    for k0 in range(0, len(extra), max_waits):
                        chunk = extra[k0 : k0 + max_waits]
                        nop = mybir.InstNoOp(
                            name=f"{inst.name}-wsplit-{k0}",
                            engine=inst.engine,
                            ins=[],
                            outs=[],
                            sync_info=mybir.SyncInfo(on_wait=chunk, on_update=[]),
                        )
                        new_list.append(nop)
                        changed = True
                    si.on_wait = keep
                new_list.append(inst)
            if changed:
                bb.instructions = new_list


def build_bass():
    nc = bass.Bass("TRN2", debug=False, num_devices=NCORES)

    def dp(nm, sh, dt=F32):
        return nc.declare_dram_parameter(nm, sh, dt, isOutput=False)

    e1aT_d = dp("e1aT", [D, H], BF16)
    e1cT_d = dp("e1cT", [D, H], BF16)
    e2q_d = [dp(f"e2q{p}", [P, 2, H], F8) for p in range(2)]
    e3q_d = [dp(f"e3q{p}", [P, 2, H], F8) for p in range(2)]
    obs_d = dp("obsT", [D, TOK], BF16)
    st_d = dp("stateT", [DS, BSH], BF16)
    n1aT_d = dp("n1aT", [FT, P, H], BF16)
    n1oT_d = dp("n1oT", [D, H], BF16)
    n1sT_d = dp("n1sT", [DS, H], BF16)
    n2T_d = dp("n2T", [FT, P, HH], BF16)
    layerT_d = dp("layerT", [DS, H], BF16)
    mu1T_d = dp("mu1T", [2 * FT, P, 256], BF16)
    s1T_d = dp("s1T", [2 * FT, P, 256], BF16)
    mu2T_d = dp("mu2T", [2, P, 128], BF16)
    s2T_d = dp("s2T", [2, P, 128], BF16)
    mu3T_d = dp("mu3T", [P, NA], BF16)
    s3T_d = dp("s3T", [P, NA], BF16)
    bias_d = dp("bias_pack", [P, 32])
    mu_d = nc.declare_dram_parameter("mu", [NA, BSH], F32, isOutput=True)
    std_d = nc.declare_dram_parameter("std", [NA, BSH], F32, isOutput=True)

    with TileContext(nc) as tc:
        with (
            tc.tile_pool(name="w", bufs=1) as wp,
            tc.tile_pool(name="act", bufs=1) as pa,
            tc.tile_pool(name="chunk", bufs=CHUNK_BUFS) as cp,
            tc.tile_pool(name="ps", bufs=6, space="PSUM") as pp,
            tc.tile_pool(name="psln", bufs=2, space="PSUM") as ppl,
        ):
            def wload(nm, dram, idx=None, dt=F32):
                src = dram[:] if idx is None else dram[idx]
                t = wp.tile(list(src.shape), dt, name=nm, tag=nm)
                nc.sync.dma_start(out=t, in_=src)
                return t

            # ---- critical-path inputs first ----
            o_all = pa.tile([D, TOK], BF16, name="o_all", tag="o_all")
            nc.sync.dma_start(out=o_all, in_=obs_d[:])
            e1aT = wload("e1aT", e1aT_d, dt=BF16)
            e1cT = wload("e1cT", e1cT_d, dt=BF16)
            bias_t = wload("bias_t", bias_d)
            st_t = pa.tile([DS, BSH], BF16, name="st_t", tag="st_t")
            nc.sync.dma_start(out=st_t, in_=st_d[:])
            e2q = [wload(f"e2q{p}", e2q_d[p], dt=F8) for p in range(2)]
            e3q = [wload(f"e3q{p}", e3q_d[p], dt=F8) for p in range(2)]
            layerw = wload("layerw", layerT_d, dt=BF16)

            def bcol(i, rows=P):
                return bias_t[0:rows, i : i + 1]

            ones_col = pa.tile([P, 1], BF16, name="ones_col", tag="ones_col")
            nc.vector.memset(ones_col, 1.0)
            ones_row = pa.tile([1, P], F32, name="ones_row", tag="ones_row")
            nc.vector.memset(ones_row, 1.0)
            eps_t = pa.tile([1, 1], F32, name="eps_t", tag="eps_t")
            nc.vector.memset(eps_t, EPS_S)

            # trigger ACT table load early (overlaps DMA wait)
            dummy_a = pa.tile([1, 1], F32, name="dummy_a", tag="dummy_a")
            nc.scalar.activation(dummy_a, eps_t, AF.Relu)

            state_bc = pa.tile([DS, TOK], BF16, name="state_bc", tag="state_bc")
            nc.vector.tensor_copy(
                state_bc[:].rearrange("s (b k) -> s b k", b=BSH),
                st_t[:, :, None].broadcast_to([DS, BSH, K]),
            )

            # PE warmup while DMAs land (HAM un-throttle + clock ramp)
            wdu = pa.tile([P, 64], BF16, name="wdu", tag="wdu")
            nc.vector.memset(wdu, 0.0)
            psd = pp.tile([64, 64], F32, name="psd", tag="ps")
            for _w in range(WARMUP_MM):
                nc.tensor.matmul(psd, wdu, wdu, start=True, stop=True)

            # ---- U/V for e1-free h1: U = A@o, V = C@o + b1 ----
            U_sb = pa.tile([P, FT, TOK], BF16, name="U_sb", tag="U_sb")
            V_sb = pa.tile([P, FT, TOK], BF16, name="V_sb", tag="V_sb")
            for m in range(FT):
                msl = slice(m * P, (m + 1) * P)
                psu = pp.tile([P, TOK], F32, name=f"psu{m}", tag="ps")
                nc.tensor.matmul(psu, e1aT[:, msl], o_all, start=True,
                                 stop=True)
                nc.vector.tensor_copy(U_sb[:, m, :], psu)
                psv = pp.tile([P, TOK], F32, name=f"psv{m}", tag="ps")
                nc.tensor.matmul(psv, e1cT[:, msl], o_all, start=True,
                                 stop=True)
                nc.scalar.activation(V_sb[:, m, :], psv, AF.Identity,
                                     bias=bcol(0 + m))

            # st_feat early (no edge deps)
            xst = []
            for m in range(FT):
                msl = slice(m * P, (m + 1) * P)
                pst = pp.tile([P, BSH], F32, name=f"pst{m}", tag="ps")
                nc.tensor.matmul(pst, layerw[:, msl], st_t, start=True,
                                 stop=True)
                xm = pa.tile([P, BSH], BF16, name=f"xst{m}", tag=f"xst{m}")
                nc.scalar.activation(xm, pst, AF.Relu, bias=bcol(16 + m))
                xst.append(xm)


            psd2 = pp.tile([64, 64], F32, name="psd2", tag="ps")
            for _w in range(WARMUP2_MM):
                nc.tensor.matmul(psd2, wdu, wdu, start=True, stop=True)

            # agg/aggn accumulators [P, FT, TOK]
            agg_all = pa.tile([P, FT, TOK], BF16, name="agg_all", tag="agg_all")
            aggn_all = pa.tile([P, FT, TOK], BF16, name="aggn_all",
                               tag="aggn_all")

            # ---- per-sample LN stats + normalize ----
            def ln_stats(b, slab, cslice, first, last):
                # accumulate ones@agg and ones@agg^2 for agg cols cslice
                n = cslice.stop - cslice.start
                o0 = cslice.start - b * K
                sq = cp.tile([P, FT, n], BF16, name="sq_s", tag="sq_s")
                nc.scalar.activation(sq, agg_all[:, :, cslice], AF.Square)
                ps_sum = slab[0:1, o0 : o0 + n]
                ps_ssq = slab[0:1, K + o0 : K + o0 + n]
                for m in range(FT):
                    nc.tensor.matmul(ps_sum, ones_col, agg_all[:, m, cslice],
                                     start=(first and m == 0),
                                     stop=(last and m == FT - 1))
                for m in range(FT):
                    nc.tensor.matmul(ps_ssq, ones_col, sq[:, m, :],
                                     start=(first and m == 0),
                                     stop=(last and m == FT - 1))

            def sample_thunks(b, norm_eng=None, slab0=None):
                """LN + node MLP for sample b as a list of emission thunks.
                Each thunk's inputs are produced by earlier thunks/chunks so
                spreading them across chunk emissions avoids FIFO bubbles."""
                bsl = slice(b * K, (b + 1) * K)
                st = {}

                def t_stats():
                    if slab0 is not None:
                        st["slab"] = slab0
                    else:
                        st["slab"] = ppl.tile([P, 512], F32, name="ln_slab",
                                              tag="ln_slab")
                        ln_stats(b, st["slab"], bsl, True, True)

                def t_mean():
                    slab = st["slab"]
                    mean_r = cp.tile([1, K], F32, name="mean_r", tag="mean_r")
                    nc.vector.tensor_scalar_mul(mean_r, slab[0:1, 0:K],
                                                1.0 / H)
                    msq_r = cp.tile([1, K], F32, name="msq_r", tag="msq_r")
                    nc.vector.tensor_mul(msq_r, mean_r, mean_r)
                    var_r = cp.tile([1, K], F32, name="var_r", tag="var_r")
                    nc.vector.scalar_tensor_tensor(
                        var_r, slab[0:1, K : 2 * K], 1.0 / H, msq_r,
                        op0=ALU.mult, op1=ALU.subtract)
                    st["mean_r"], st["var_r"] = mean_r, var_r

                def t_rstd():
                    lnv_r = cp.tile([1, K], F32, name="lnv_r", tag="lnv_r")
                    nc.scalar.activation(lnv_r, st["var_r"], AF.Ln,
                                         bias=eps_t)
                    rstd_r = cp.tile([1, K], F32, name="rstd_r", tag="rstd_r")
                    nc.scalar.activation(rstd_r, lnv_r, AF.Exp, scale=-0.5)
                    st["rstd_r"] = rstd_r

                def t_bcast():
                    slab = st["slab"]
                    ps_mb = slab[:, 2 * K : 3 * K]
                    nc.tensor.matmul(ps_mb, ones_row, st["mean_r"],
                                     start=True, stop=True)
                    ps_rb = slab[:, 3 * K : 4 * K]
                    nc.tensor.matmul(ps_rb, ones_row, st["rstd_r"],
                                     start=True, stop=True)
                    mean_bc = cp.tile([P, K], F32, name="mean_bc",
                                      tag="mean_bc")
                    nc.scalar.copy(mean_bc, ps_mb)
                    rstd_bc = cp.tile([P, K], F32, name="rstd_bc",
                                      tag="rstd_bc")
                    nc.scalar.copy(rstd_bc, ps_rb)
                    st["mean_bc"], st["rstd_bc"] = mean_bc, rstd_bc

                def t_norm():
                    eng = norm_eng or nc.gpsimd
                    tmp = cp.tile([P, FT, K], BF16, name="ln_tmp",
                                  tag="ln_tmp")
                    eng.tensor_tensor(
                        tmp, agg_all[:, :, bsl],
                        st["mean_bc"][:, None, :].broadcast_to([P, FT, K]),
                        op=ALU.subtract)
                    eng.tensor_tensor(
                        aggn_all[:, :, bsl], tmp,
                        st["rstd_bc"][:, None, :].broadcast_to([P, FT, K]),
                        op=ALU.mult)

                def t_n1(m):
                    def f():
                        slab = st["slab"]
                        msl = slice(m * P, (m + 1) * P)
                        psn = slab[:, 256 + K * m : 256 + K * (m + 1)]
                        nc.tensor.matmul(psn, n1ow_t[0][:, msl],
                                         o_all[:, bsl],
                                         start=True, stop=False)
                        nc.tensor.matmul(psn, n1sw_t[0][:, msl],
                                         state_bc[:, bsl],
                                         start=False, stop=False)
                        for k2 in range(FT):
                            nc.tensor.matmul(psn, n1aw[k2][:, msl],
                                             aggn_all[:, k2, bsl],
                                             start=False,
                                             stop=(k2 == FT - 1))
                        nc.scalar.activation(hn1[m][:, bsl], psn, AF.Relu,
                                             bias=bcol(12 + m))
                    return f

                def t_n2(m2):
                    def f():
                        slab = st["slab"]
                        msl = slice(m2 * P, (m2 + 1) * P)
                        psn2 = slab[:, K * m2 : K * (m2 + 1)]
                        for k2 in range(FT):
                            nc.tensor.matmul(psn2, n2w[k2][:, msl],
                                             hn1[k2][:, bsl],
                                             start=(k2 == 0),
                                             stop=(k2 == FT - 1))
                        eng = nc.vector
                        with nc.allow_low_precision(reason="bf16 pool"):
                            eng.reduce_sum(
                                out=pool_sum[m2][:, b : b + 1],
                                in_=psn2[:, None, :], axis=AX.X)
                            eng.reduce_max(
                                out=pool_max[m2][:, b : b + 1],
                                in_=psn2[:, None, :], axis=AX.X)
                    return f

                return ([t_stats, t_mean, t_rstd, t_bcast, t_norm]
                        + [t_n1(m) for m in range(FT)]
                        + [t_n2(m2) for m2 in range(HH // P)])

            hn1 = []
            for m in range(FT):
                hn1.append(pa.tile([P, TOK], BF16, name=f"hn1_{m}",
                                   tag=f"hn1_{m}"))
            pool_sum, pool_max = [], []
            for m2 in range(HH // P):
                pool_sum.append(pa.tile([P, BSH], BF16, name=f"pool_s{m2}",
                                        tag=f"pool_s{m2}"))
                pool_max.append(pa.tile([P, BSH], BF16, name=f"pool_m{m2}",
                                        tag=f"pool_m{m2}"))

            # ---------------- edge MLP over K x K pairs ----------------
            chunk_list = [(b, ib) for b in range(BSH) for ib in range(NCH)]
            h1ts = {}
            t_pre = {}

            def emit_h1_add(ci):
                b, ib = chunk_list[ci]
                t = cp.tile([P, FT, IBLK * K], BF16, name="t_pre", tag="t_pre")
                bsl = slice(b * K, (b + 1) * K)
                usl = slice(b * K + ib * IBLK, b * K + (ib + 1) * IBLK)
                for q in range(2):
                    msl = slice(2 * q, 2 * q + 2)
                    nc.gpsimd.tensor_tensor(
                        t[:, msl, :].rearrange("p m (i j) -> p m i j", i=IBLK),
                        V_sb[:, msl, None, bsl].broadcast_to(
                            [P, 2, IBLK, K]),
                        U_sb[:, msl, usl, None].broadcast_to(
                            [P, 2, IBLK, K]),
                        op=ALU.add)
                t_pre[ci] = t

            def emit_h1_cast(ci):
                t = t_pre.pop(ci)
                h1t = cp.tile([P, FT, IBLK * K], F8, name="h1t", tag="h1t")
                for q in range(2):
                    msl = slice(2 * q, 2 * q + 2)
                    nc.scalar.activation(h1t[:, msl, :], t[:, msl, :], AF.Relu)
                h1ts[ci] = h1t

            NTH = (IBLK * K) // MM_FREE

            slabs = {}
            thunk_q = []
            n1aw, n1ow_t, n1sw_t, n2w = [], [], [], []
            emit_h1_add(0)
            emit_h1_cast(0)
            if len(chunk_list) > 1:
                emit_h1_add(1)
                emit_h1_cast(1)
            if len(chunk_list) > 2:
                emit_h1_add(2)
            pending_red = None
            for ci, (b, ib) in enumerate(chunk_list):
                c0 = b * K + ib * IBLK
                h1t = h1ts.pop(ci)

                # e2: fp8 DR, h2 -> fp8
                h2t = cp.tile([P, FT, IBLK * K], F8, name="h2t", tag="h2t")
                ps2 = []
                for m in range(FT):
                    msl = slice(m * P, (m + 1) * P)
                    p2 = pp.tile([P, IBLK * K], F32, name=f"ps2_{m}", tag="ps")
                    for th in range(NTH):
                        tsl = slice(th * MM_FREE, (th + 1) * MM_FREE)
                        for si in range(2):
                            nc.tensor.matmul(
                                p2[:, tsl], e2q[si][:, :, msl],
                                h1t[:, 2 * si : 2 * si + 2, tsl],
                                start=(si == 0), stop=(si == 1),
                                perf_mode=DR)
                    ps2.append(p2)
                for m in range(FT):
                    if (m % 2 == 0) if N_H2_ACT == 2 else (m < N_H2_ACT):
                        nc.scalar.activation(h2t[:, m, :], ps2[m], AF.Relu,
                                             bias=bcol(4 + m))
                    else:
                        nc.vector.tensor_scalar(
                            h2t[:, m, :], ps2[m], bcol(4 + m), 0.0,
                            op0=ALU.add, op1=ALU.max)

                if ci + 3 < len(chunk_list):
                    emit_h1_add(ci + 3)
                if ci + 2 < len(chunk_list):
                    emit_h1_cast(ci + 2)

                # e3: fp8 DR; h3 evict + j-sum into agg_all
                h3t = cp.tile([P, FT, IBLK * K], BF16, name="h3t", tag="h3t")
                for m in range(FT):
                    msl = slice(m * P, (m + 1) * P)
                    p3 = pp.tile([P, IBLK * K], F32, name=f"ps3_{m}", tag="ps")
                    for th in range(NTH):
                        tsl = slice(th * MM_FREE, (th + 1) * MM_FREE)
                        for si in range(2):
                            nc.tensor.matmul(
                                p3[:, tsl], e3q[si][:, :, msl],
                                h2t[:, 2 * si : 2 * si + 2, tsl],
                                start=(si == 0), stop=(si == 1),
                                perf_mode=DR)
                    if (m % 2 == 0) if N_H3_ACT == 2 else (m < N_H3_ACT):
                        nc.scalar.activation(h3t[:, m, :], p3, AF.Relu,
                                             bias=bcol(8 + m))
                    else:
                        nc.vector.tensor_scalar(
                            h3t[:, m, :], p3, bcol(8 + m), 0.0,
                            op0=ALU.add, op1=ALU.max)
                # deferred one-instr reduce for previous chunk
                if pending_red is not None:
                    pr_h3, pr_c0 = pending_red
                    with nc.allow_low_precision(reason="bf16 agg"):
                        nc.vector.reduce_sum(
                            out=agg_all[:, :, pr_c0 : pr_c0 + IBLK],
                            in_=pr_h3[:].rearrange("p m (i j) -> p m i j",
                                                   i=IBLK),
                            axis=AX.X)
                pending_red = (h3t, c0)
                if ci == 1:
                    # node weights: DMA streams during early chunks
                    for k in range(FT):
                        n1aw.append(wload(f"n1aw{k}", n1aT_d, k, BF16))
                    n1ow_t.append(wload("n1ow", n1oT_d, dt=BF16))
                    n1sw_t.append(wload("n1sw", n1sT_d, dt=BF16))
                    for k in range(FT):
                        n2w.append(wload(f"n2w{k}", n2T_d, k, BF16))
                if LN_INLINE and ci > 0 and ci % NCH == 0:
                    thunk_q.extend(sample_thunks(ci // NCH - 1))
                for _ in range(THUNKS_PC):
                    if thunk_q:
                        thunk_q.pop(0)()
                if LN_INLINE and INC_STATS and ci >= (BSH - 1) * NCH + 1:
                    # incremental stats for the last sample's landed agg cols
                    if ci == (BSH - 1) * NCH + 1:
                        last_slab = ppl.tile([P, 512], F32, name="ln_slab",
                                             tag="ln_slab")
                        slabs["last"] = last_slab
                    pc0 = (BSH - 1) * K + (ci - 1 - (BSH - 1) * NCH) * IBLK
                    ln_stats(BSH - 1, slabs["last"],
                             slice(pc0, pc0 + IBLK), first=(pc0 % K == 0),
                             last=False)

            pr_h3, pr_c0 = pending_red
            with nc.allow_low_precision(reason="bf16 agg"):
                for m in range(FT):
                    nc.vector.reduce_sum(
                        out=agg_all[:, m, pr_c0 : pr_c0 + IBLK],
                        in_=pr_h3[:, m, :].rearrange("p (i j) -> p i j",
                                                     i=IBLK),
                        axis=AX.X)
            while thunk_q:
                thunk_q.pop(0)()
            if LN_INLINE:
                if INC_STATS:
                    ln_stats(BSH - 1, slabs["last"],
                             slice(pr_c0, pr_c0 + IBLK), first=False,
                             last=True)
                    ths = sample_thunks(BSH - 1, norm_eng=nc.vector,
                                        slab0=slabs["last"])
                else:
                    ths = sample_thunks(BSH - 1, norm_eng=nc.vector)
                # node_pre before the LN scalar chain to hide its latency
                for t in ths:
                    t()
            else:
                for b in range(BSH):
                    for t in sample_thunks(b):
                        t()

            # ---- head weights (loads emitted late; DMA overlaps edge) ----
            mu1w = [wload(f"mu1w{k}", mu1T_d, k, BF16) for k in range(2 * FT)]
            s1w = [wload(f"s1w{k}", s1T_d, k, BF16) for k in range(2 * FT)]
            mu2w = [wload(f"mu2w{k}", mu2T_d, k, BF16) for k in range(2)]
            s2w = [wload(f"s2w{k}", s2T_d, k, BF16) for k in range(2)]
            mu3w = wload("mu3w", mu3T_d, dt=BF16)
            s3w = wload("s3w", s3T_d, dt=BF16)

            xs = xst + pool_sum + pool_max

            hl1 = {"mu": [], "s": []}
            for tag, w1, bc1 in (("mu", mu1w, 20), ("s", s1w, 24)):
                for m in range(2):
                    msl = slice(m * P, (m + 1) * P)
                    ph = pp.tile([P, BSH], F32, name=f"p{tag}1_{m}", tag="ps")
                    for k2 in range(2 * FT):
                        nc.tensor.matmul(ph, w1[k2][:, msl], xs[k2],
                                         start=(k2 == 0),
                                         stop=(k2 == 2 * FT - 1))
                    hm = pa.tile([P, BSH], BF16, name=f"h{tag}1_{m}",
                                 tag=f"h{tag}1_{m}")
                    eng = nc.scalar if tag == "mu" else None
                    if eng is not None:
                        eng.activation(hm, ph, AF.Relu, bias=bcol(bc1 + m))
                    else:
                        nc.vector.tensor_scalar(hm, ph, bcol(bc1 + m), 0.0,
                                                op0=ALU.add, op1=ALU.max)
                    hl1[tag].append(hm)
            hm2 = {}
            for tag, w2, bc2 in (("mu", mu2w, 22), ("s", s2w, 26)):
                ph2 = pp.tile([P, BSH], F32, name=f"p{tag}2", tag="ps")
                for k2 in range(2):
                    nc.tensor.matmul(ph2, w2[k2], hl1[tag][k2],
                                     start=(k2 == 0), stop=(k2 == 1))
                h2t_ = pa.tile([P, BSH], BF16, name=f"h{tag}2", tag=f"h{tag}2")
                if tag == "mu":
                    nc.scalar.activation(h2t_, ph2, AF.Relu, bias=bcol(bc2))
                else:
                    nc.vector.tensor_scalar(h2t_, ph2, bcol(bc2), 0.0,
                                            op0=ALU.add, op1=ALU.max)
                hm2[tag] = h2t_
            ph3_mu = pp.tile([NA, BSH], F32, name="pmu3", tag="ps")
            nc.tensor.matmul(ph3_mu, mu3w, hm2["mu"], start=True, stop=True)
            ph3_s = pp.tile([NA, BSH], F32, name="ps3h", tag="ps")
            nc.tensor.matmul(ph3_s, s3w, hm2["s"], start=True, stop=True)

            mu_sb = pa.tile([NA, BSH], F32, name="mu_sb", tag="mu_sb")
            nc.scalar.activation(mu_sb, ph3_mu, AF.Identity,
                                 bias=bcol(23, rows=NA))
            nc.sync.dma_start(out=mu_d[:], in_=mu_sb)
            std_sb = pa.tile([NA, BSH], F32, name="std_sb", tag="std_sb")
            nc.vector.tensor_scalar(std_sb, ph3_s, bcol(27, rows=NA), 0.0,
                                    op0=ALU.add, op1=ALU.bypass)
            nc.sync.dma_start(out=std_d[:], in_=std_sb)

    _split_excess_waits(nc)
    return nc


def _q8(x):
    import ml_dtypes
    return np.asarray(x, np.float32).astype(ml_dtypes.float8_e4m3)


def _f8f(x):
    return _q8(x).astype(np.float32)


def prep_weights(inp):
    """Host-side weight prep -> dict of replicated arrays."""
    import ml_dtypes
    bf = ml_dtypes.bfloat16

    def fb(a):
        return np.ascontiguousarray(np.asarray(a, np.float32), dtype=bf)

    e1_w = np.asarray(inp["e1_w"], np.float32)
    e2_w = np.asarray(inp["e2_w"], np.float32)
    e3_w = np.asarray(inp["e3_w"], np.float32)
    n1_w = np.asarray(inp["n1_w"], np.float32)
    ln_g = np.asarray(inp["ln_g"], np.float32)
    ln_b = np.asarray(inp["ln_b"], np.float32)
    n2_b = np.asarray(inp["n2_b"], np.float32)
    mu1_w = np.asarray(inp["mu1_w"], np.float32)
    s1_w = np.asarray(inp["s1_w"], np.float32)
    b1 = np.asarray(inp["e1_b"], np.float32)
    b2 = np.asarray(inp["e2_b"], np.float32)
    b3 = np.asarray(inp["e3_b"], np.float32)

    d = {}
    A_ = e1_w[:, :D]
    C_ = e1_w[:, D:]
    d["e1aT"] = fb(A_.T)
    d["e1cT"] = fb(C_.T)

    def pack_dr(wt):
        out = {}
        for p in range(2):
            arr = np.zeros((P, 2, H), np.float32)
            for q_ in range(2):
                ks = (2 * p + q_) * P
                arr[:, q_, :] = wt[ks : ks + P, :]
            out[p] = _q8(arr)
        return out

    w2t = e2_w.T
    q2 = _f8f(w2t)
    for p, a in pack_dr(q2).items():
        d[f"e2q{p}"] = a
    e2_eff = q2.T

    w3t = e3_w.T
    q3 = _f8f(w3t)
    for p, a in pack_dr(q3).items():
        d[f"e3q{p}"] = a
    e3_eff = q3.T

    # ---- bias corrections via subsampled calibration means ----
    obs = np.asarray(inp["obs"], np.float32)
    o = obs.transpose(0, 2, 1).reshape(B * K, D)
    obf = np.asarray(np.asarray(o, np.float32).astype(bf), np.float32)
    Abf = np.asarray(fb(A_), np.float32)
    Cbf = np.asarray(fb(C_), np.float32)
    U = (obf @ Abf.T).reshape(B, K, H)
    V = (obf @ Cbf.T).reshape(B, K, H)
    jsub = np.arange(0, K, 8)
    m1 = np.zeros(H, np.float64)
    m2 = np.zeros(H, np.float64)
    for bi in range(B):
        h1 = np.maximum(U[bi][:, None, :] + V[bi][None, jsub, :] + b1, 0.0)
        h1 = _f8f(h1.reshape(-1, H))
        m1 += h1.mean(0) / B
        h2 = np.maximum(h1 @ e2_w.T + b2, 0.0)
        m2 += h2.mean(0) / B
    m1 = m1.astype(np.float32)
    m2 = m2.astype(np.float32)
    b2_eff = b2 - (e2_eff - e2_w) @ m1
    b3_eff = b3 - (e3_eff - e3_w) @ m2

    d["n1aT"] = fb((n1_w[:, D : D + H] * ln_g[None, :]).T.reshape(FT, P, H))
    d["n1oT"] = fb(n1_w[:, :D].T)
    d["n1sT"] = fb(n1_w[:, D + H :].T)
    d["n2T"] = fb(np.asarray(inp["n2_w"], np.float32).T.reshape(FT, P, HH))
    d["layerT"] = fb(np.asarray(inp["layer_w"], np.float32).T)

    mu1 = mu1_w.copy()
    mu1[:, H : H + HH] *= 1.0 / K
    d["mu1T"] = fb(mu1.T.reshape(2 * FT, P, 256))
    s1 = s1_w.copy()
    s1[:, H : H + HH] *= 1.0 / K
    d["s1T"] = fb(s1.T.reshape(2 * FT, P, 256))
    d["mu2T"] = fb(np.asarray(inp["mu2_w"], np.float32).T.reshape(2, P, 128))
    d["s2T"] = fb(np.asarray(inp["s2_w"], np.float32).T.reshape(2, P, 128))
    d["mu3T"] = fb(np.asarray(inp["mu3_w"], np.float32).T)
    d["s3T"] = fb(np.asarray(inp["s3_w"], np.float32).T)

    n1_b_eff = np.asarray(inp["n1_b"], np.float32) + n1_w[:, D : D + H] @ ln_b
    mu1_b_eff = (np.asarray(inp["mu1_b"], np.float32)
                 + (mu1_w[:, H : H + HH] + mu1_w[:, H + HH :]) @ n2_b)
    s1_b_eff = (np.asarray(inp["s1_b"], np.float32)
                + (s1_w[:, H : H + HH] + s1_w[:, H + HH :]) @ n2_b)

    bp = np.zeros((P, 32), np.float32)
    bp[:, 0:4] = b1.reshape(FT, P).T
    bp[:, 4:8] = b2_eff.reshape(FT, P).T
    bp[:, 8:12] = b3_eff.reshape(FT, P).T
    bp[:, 12:16] = n1_b_eff.reshape(FT, P).T
    bp[:, 16:20] = np.asarray(inp["layer_b"], np.float32).reshape(FT, P).T
    bp[:, 20:22] = mu1_b_eff.reshape(2, P).T
    bp[:, 22] = np.asarray(inp["mu2_b"], np.float32)
    bp[0:NA, 23] = np.asarray(inp["mu3_b"], np.float32)
    bp[:, 24:26] = s1_b_eff.reshape(2, P).T
    bp[:, 26] = np.asarray(inp["s2_b"], np.float32)
    bp[0:NA, 27] = np.asarray(inp["s3_b"], np.float32)
    d["bias_pack"] = bp
    return d


def make_in_maps(inputs):
    import ml_dtypes
    bf = ml_dtypes.bfloat16
    w = prep_weights(inputs)
    obs = np.asarray(inputs["obs"], np.float32)
    state = np.asarray(inputs["state"], np.float32)
    in_maps = []
    for c in range(NCORES):
        m = dict(w)
        ob = obs[c * BSH : (c + 1) * BSH]          # [BSH, D, K]
        m["obsT"] = np.ascontiguousarray(
            ob.transpose(1, 0, 2).reshape(D, TOK), dtype=bf)
        m["stateT"] = np.ascontiguousarray(
            state[c * BSH : (c + 1) * BSH].T, dtype=bf)
        in_maps.append(m)
    return in_maps


_NC_CACHE = {}


def get_nc():
    key = (WARMUP_MM, WARMUP2_MM, CHUNK_BUFS, N_H2_ACT, N_H3_ACT, MM_FREE,
           LN_INLINE, THUNKS_PC, INC_STATS)
    if key not in _NC_CACHE:
        _NC_CACHE[key] = build_bass()
    return _NC_CACHE[key]


def run(in_maps, trace=False, **kw):
    nc = get_nc()
    return run_bass_kernel_spmd(nc, in_maps, core_ids=list(range(NCORES)),
                                trace=trace, **kw)


def gather(res_list):
    mu = np.concatenate([r["mu"].T for r in res_list], axis=0)
    pre = np.concatenate([r["std"].T for r in res_list],
                         axis=0).astype(np.float64)
    std = np.clip(np.log1p(np.exp(pre)) + 0.001, 0.1, 2.0)
    return mu.astype(np.float32), std.astype(np.float32)


def kernel(**inputs):
    res = run(make_in_maps(inputs))
    return gather(res.results)


# revision 4
# speedup vs baseline: 1.0101x; 1.0017x over previous
"""Trainium2 Bass kernel (v20) for nn_ActorNetwork (GNN message passing actor).

Self-contained: hardcodes shapes B=32, K=64, D=4, DS=4, H=512, HH=256, NA=2.
Data-parallel over batch across 8 NeuronCores (4 samples/core).

Structure (per core, per 512-edge-token chunk):
- h1 = relu(U_i + V_j + b1): U=A@o, V=C@o+b1 once per core (PE); per chunk
  Pool broadcast-add (bf16) + ACT relu-cast to fp8.
- e2/e3: fp8 DoubleRow matmuls on PE (the bottleneck engine by design).
- h2/h3 PSUM evictions split ACT/DVE; j-reduction as one DVE instr per
  chunk into a bf16 agg_all tile.
- LN stats+normalize per sample, overlapped into the edge stream
  (normalize on Pool); node MLP + pools + heads in the tail with
  full-width instructions.
"""
import os as _os

import numpy as np

import concourse.bass as bass
import concourse.mybir as mybir
from concourse.bass_utils import run_bass_kernel_spmd
from concourse.tile import TileContext

# ---- problem constants ----
B, K, D, DS, H, HH, NA = 32, 64, 4, 4, 512, 256, 2
NCORES = 8
BSH = B // NCORES            # samples per core = 4
P = 128
FT = H // P                  # 4 feature tiles of hidden dim
TOK = BSH * K                # 256 node tokens per core
IBLK = 8                     # i-rows per edge chunk (8*64 = 512 tokens)
NCH = K // IBLK              # 8 chunks per sample
ET = K * K                   # 4096 edge tokens per sample

F32 = mybir.dt.float32
BF16 = mybir.dt.bfloat16
F8 = mybir.dt.float8e4
AF = mybir.ActivationFunctionType
AX = mybir.AxisListType
ALU = mybir.AluOpType
DR = mybir.MatmulPerfMode.DoubleRow

WARMUP_MM = int(_os.environ.get("K20_WARMUP_MM", "16"))
WARMUP2_MM = int(_os.environ.get("K20_WARMUP2_MM", "48"))
CHUNK_BUFS = int(_os.environ.get("K20_CHUNK_BUFS", "3"))
N_H2_ACT = int(_os.environ.get("K20_H2_ACT", "2"))   # h2 evicts on ACT (rest DVE)
N_H3_ACT = int(_os.environ.get("K20_H3_ACT", "2"))   # h3 evicts on ACT (rest DVE)
MM_FREE = int(_os.environ.get("K20_MM_FREE", "512"))  # moving cols per DR matmul
LN_INLINE = int(_os.environ.get("K20_LN_INLINE", "1"))  # per-sample LN in edge
THUNKS_PC = int(_os.environ.get("K20_THUNKS_PC", "2"))  # ln/node thunks per chunk
INC_STATS = int(_os.environ.get("K20_INC_STATS", "1"))  # incremental stats for last sample

EPS_S = (K * K) * 1e-5


def _split_excess_waits(nc, max_waits=1):
    """walrus in this container rejects >~2 sem waits on one instruction."""
    for f in nc.m.functions:
        for bb in f.blocks:
            insts = list(bb.instructions)
            new_list = []
            changed = False
            for inst in insts:
                si = inst.sync_info
                if si is not None and si.on_wait and len(si.on_wait) > max_waits:
                    waits = list(si.on_wait)
                    extra, keep = waits[:-max_waits], waits[-max_waits:]
                    for k0 in range(0, len(extra), max_waits):
                        chunk = extra[k0 : k0 + max_waits]
                        nop = mybir.InstNoOp(
                            name=f"{inst.name}-wsplit-{k0}",
                            engine=inst.engine,
                            ins=[],
                            outs=[],
                            sync_info=mybir.SyncInfo(on_wait=chunk, on_update=[]),
                        )
                        new_list.append(nop)
                        changed = True
                    si.on_wait = keep
                new_list.append(inst)
            if changed:
                bb.instructions = new_list


def build_bass():
    nc = bass.Bass("TRN2", debug=False, num_devices=NCORES)

    def dp(nm, sh, dt=F32):
        return nc.declare_dram_parameter(nm, sh, dt, isOutput=False)

    e1aT_d = dp("e1aT", [D, H], BF16)
    e1cT_d = dp("e1cT", [D, H], BF16)
    e2q_d = [dp(f"e2q{p}", [P, 2, H], F8) for p in range(2)]
    e3q_d = [dp(f"e3q{p}", [P, 2, H], F8) for p in range(2)]
    obs_d = dp("obsT", [D, TOK], BF16)
    st_d = dp("stateT", [DS, BSH], BF16)
    n1aT_d = dp("n1aT", [FT, P, H], BF16)
    n1oT_d = dp("n1oT", [D, H], BF16)
    n1sT_d = dp("n1sT", [DS, H], BF16)
    n2T_d = dp("n2T", [FT, P, HH], BF16)
    layerT_d = dp("layerT", [DS, H], BF16)
    mu1T_d = dp("mu1T", [2 * FT, P, 256], BF16)
    s1T_d = dp("s1T", [2 * FT, P, 256], BF16)
    mu2T_d = dp("mu2T", [2, P, 128], BF16)
    s2T_d = dp("s2T", [2, P, 128], BF16)
    mu3T_d = dp("mu3T", [P, NA], BF16)
    s3T_d = dp("s3T", [P, NA], BF16)
    bias_d = dp("bias_pack", [P, 32])
    mu_d = nc.declare_dram_parameter("mu", [NA, BSH], F32, isOutput=True)
    std_d = nc.declare_dram_parameter("std", [NA, BSH], F32, isOutput=True)

    with TileContext(nc) as tc:
        with (
            tc.tile_pool(name="w", bufs=1) as wp,
            tc.tile_pool(name="act", bufs=1) as pa,
            tc.tile_pool(name="chunk", bufs=CHUNK_BUFS) as cp,
            tc.tile_pool(name="ps", bufs=6, space="PSUM") as pp,
            tc.tile_pool(name="psln", bufs=2, space="PSUM") as ppl,
        ):
            def wload(nm, dram, idx=None, dt=F32):
                src = dram[:] if idx is None else dram[idx]
                t = wp.tile(list(src.shape), dt, name=nm, tag=nm)
                nc.sync.dma_start(out=t, in_=src)
                return t

            # ---- critical-path inputs first ----
            o_all = pa.tile([D, TOK], BF16, name="o_all", tag="o_all")
            nc.sync.dma_start(out=o_all, in_=obs_d[:])
            e1aT = wload("e1aT", e1aT_d, dt=BF16)
            e1cT = wload("e1cT", e1cT_d, dt=BF16)
            bias_t = wload("bias_t", bias_d)
            st_t = pa.tile([DS, BSH], BF16, name="st_t", tag="st_t")
            nc.sync.dma_start(out=st_t, in_=st_d[:])
            e2q = [wload(f"e2q{p}", e2q_d[p], dt=F8) for p in range(2)]
            e3q = [wload(f"e3q{p}", e3q_d[p], dt=F8) for p in range(2)]
            layerw = wload("layerw", layerT_d, dt=BF16)

            def bcol(i, rows=P):
                return bias_t[0:rows, i : i + 1]

            ones_col = pa.tile([P, 1], BF16, name="ones_col", tag="ones_col")
            nc.vector.memset(ones_col, 1.0)
            ones_row = pa.tile([1, P], F32, name="ones_row", tag="ones_row")
            nc.vector.memset(ones_row, 1.0)
            eps_t = pa.tile([1, 1], F32, name="eps_t", tag="eps_t")
            nc.vector.memset(eps_t, EPS_S)

            # trigger ACT table load early (overlaps DMA wait)
            dummy_a = pa.tile([1, 1], F32, name="dummy_a", tag="dummy_a")
            nc.scalar.activation(dummy_a, eps_t, AF.Relu)

            state_bc = pa.tile([DS, TOK], BF16, name="state_bc", tag="state_bc")
            nc.vector.tensor_copy(
                state_bc[:].rearrange("s (b k) -> s b k", b=BSH),
                st_t[:, :, None].broadcast_to([DS, BSH, K]),
            )

            # PE warmup while DMAs land (HAM un-throttle + clock ramp)
            wdu = pa.tile([P, 64], BF16, name="wdu", tag="wdu")
            nc.vector.memset(wdu, 0.0)
            psd = pp.tile([64, 64], F32, name="psd", tag="ps")
            for _w in range(WARMUP_MM):
                nc.tensor.matmul(psd, wdu, wdu, start=True, stop=True)

            # ---- U/V for e1-free h1: U = A@o, V = C@o + b1 ----
            U_sb = pa.tile([P, FT, TOK], BF16, name="U_sb", tag="U_sb")
            V_sb = pa.tile([P, FT, TOK], BF16, name="V_sb", tag="V_sb")
            for m in range(FT):
                msl = slice(m * P, (m + 1) * P)
                psu = pp.tile([P, TOK], F32, name=f"psu{m}", tag="ps")
                nc.tensor.matmul(psu, e1aT[:, msl], o_all, start=True,
                                 stop=True)
                nc.vector.tensor_copy(U_sb[:, m, :], psu)
                psv = pp.tile([P, TOK], F32, name=f"psv{m}", tag="ps")
                nc.tensor.matmul(psv, e1cT[:, msl], o_all, start=True,
                                 stop=True)
                nc.scalar.activation(V_sb[:, m, :], psv, AF.Identity,
                                     bias=bcol(0 + m))

            # st_feat early (no edge deps)
            xst = []
            for m in range(FT):
                msl = slice(m * P, (m + 1) * P)
                pst = pp.tile([P, BSH], F32, name=f"pst{m}", tag="ps")
                nc.tensor.matmul(pst, layerw[:, msl], st_t, start=True,
                                 stop=True)
                xm = pa.tile([P, BSH], BF16, name=f"xst{m}", tag=f"xst{m}")
                nc.scalar.activation(xm, pst, AF.Relu, bias=bcol(16 + m))
                xst.append(xm)


            psd2 = pp.tile([64, 64], F32, name="psd2", tag="ps")
            for _w in range(WARMUP2_MM):
                nc.tensor.matmul(psd2, wdu, wdu, start=True, stop=True)

            # agg/aggn accumulators [P, FT, TOK]
            agg_all = pa.tile([P, FT, TOK], BF16, name="agg_all", tag="agg_all")
            aggn_all = pa.tile([P, FT, TOK], BF16, name="aggn_all",
                               tag="aggn_all")

            # ---- per-sample LN stats + normalize ----
            def ln_stats(b, slab, cslice, first, last):
                # accumulate ones@agg and ones@agg^2 for agg cols cslice
                n = cslice.stop - cslice.start
                o0 = cslice.start - b * K
                sq = cp.tile([P, FT, n], BF16, name="sq_s", tag="sq_s")
                nc.scalar.activation(sq, agg_all[:, :, cslice], AF.Square)
                ps_sum = slab[0:1, o0 : o0 + n]
                ps_ssq = slab[0:1, K + o0 : K + o0 + n]
                for m in range(FT):
                    nc.tensor.matmul(ps_sum, ones_col, agg_all[:, m, cslice],
                                     start=(first and m == 0),
                                     stop=(last and m == FT - 1))
                for m in range(FT):
                    nc.tensor.matmul(ps_ssq, ones_col, sq[:, m, :],
                                     start=(first and m == 0),
                                     stop=(last and m == FT - 1))

            def sample_thunks(b, norm_eng=None, slab0=None):
                """LN + node MLP for sample b as a list of emission thunks.
                Each thunk's inputs are produced by earlier thunks/chunks so
                spreading them across chunk emissions avoids FIFO bubbles."""
                bsl = slice(b * K, (b + 1) * K)
                st = {}

                def t_stats():
                    if slab0 is not None:
                        st["slab"] = slab0
                    else:
                        st["slab"] = ppl.tile([P, 512], F32, name="ln_slab",
                                              tag="ln_slab")
                        ln_stats(b, st["slab"], bsl, True, True)

                def t_mean():
                    slab = st["slab"]
                    mean_r = cp.tile([1, K], F32, name="mean_r", tag="mean_r")
                    nc.vector.tensor_scalar_mul(mean_r, slab[0:1, 0:K],
                                                1.0 / H)
                    msq_r = cp.tile([1, K], F32, name="msq_r", tag="msq_r")
                    nc.vector.tensor_mul(msq_r, mean_r, mean_r)
                    var_r = cp.tile([1, K], F32, name="var_r", tag="var_r")
                    nc.vector.scalar_tensor_tensor(
                        var_r, slab[0:1, K : 2 * K], 1.0 / H, msq_r,
                        op0=ALU.mult, op1=ALU.subtract)
                    st["mean_r"], st["var_r"] = mean_r, var_r

                def t_rstd():
                    lnv_r = cp.tile([1, K], F32, name="lnv_r", tag="lnv_r")
                    nc.scalar.activation(lnv_r, st["var_r"], AF.Ln,
                                         bias=eps_t)
                    rstd_r = cp.tile([1, K], F32, name="rstd_r", tag="rstd_r")
                    nc.scalar.activation(rstd_r, lnv_r, AF.Exp, scale=-0.5)
                    st["rstd_r"] = rstd_r

                def t_bcast():
                    slab = st["slab"]
                    ps_mb = slab[:, 2 * K : 3 * K]
                    nc.tensor.matmul(ps_mb, ones_row, st["mean_r"],
                                     start=True, stop=True)
                    ps_rb = slab[:, 3 * K : 4 * K]
                    nc.tensor.matmul(ps_rb, ones_row, st["rstd_r"],
                                     start=True, stop=True)
                    mean_bc = cp.tile([P, K], F32, name="mean_bc",
                                      tag="mean_bc")
                    nc.scalar.copy(mean_bc, ps_mb)
                    rstd_bc = cp.tile([P, K], F32, name="rstd_bc",
                                      tag="rstd_bc")
                    nc.scalar.copy(rstd_bc, ps_rb)
                    st["mean_bc"], st["rstd_bc"] = mean_bc, rstd_bc

                def t_norm():
                    eng = norm_eng or nc.gpsimd
                    tmp = cp.tile([P, FT, K], BF16, name="ln_tmp",
                                  tag="ln_tmp")
                    eng.tensor_tensor(
                        tmp, agg_all[:, :, bsl],
                        st["mean_bc"][:, None, :].broadcast_to([P, FT, K]),
                        op=ALU.subtract)
                    eng.tensor_tensor(
                        aggn_all[:, :, bsl], tmp,
                        st["rstd_bc"][:, None, :].broadcast_to([P, FT, K]),
                        op=ALU.mult)

                def t_n1(m):
                    def f():
                        slab = st["slab"]
                        msl = slice(m * P, (m + 1) * P)
                        psn = slab[:, 256 + K * m : 256 + K * (m + 1)]
                        nc.tensor.matmul(psn, n1ow_t[0][:, msl],
                                         o_all[:, bsl],
                                         start=True, stop=False)
                        nc.tensor.matmul(psn, n1sw_t[0][:, msl],
                                         state_bc[:, bsl],
                                         start=False, stop=False)
                        for k2 in range(FT):
                            nc.tensor.matmul(psn, n1aw[k2][:, msl],
                                             aggn_all[:, k2, bsl],
                                             start=False,
                                             stop=(k2 == FT - 1))
                        nc.scalar.activation(hn1[m][:, bsl], psn, AF.Relu,
                                             bias=bcol(12 + m))
                    return f

                def t_n2(m2):
                    def f():
                        slab = st["slab"]
                        msl = slice(m2 * P, (m2 + 1) * P)
                        psn2 = slab[:, K * m2 : K * (m2 + 1)]
                        for k2 in range(FT):
                            nc.tensor.matmul(psn2, n2w[k2][:, msl],
                                             hn1[k2][:, bsl],
                                             start=(k2 == 0),
                                             stop=(k2 == FT - 1))
                        eng = nc.vector
                        with nc.allow_low_precision(reason="bf16 pool"):
                            eng.reduce_sum(
                                out=pool_sum[m2][:, b : b + 1],
                                in_=psn2[:, None, :], axis=AX.X)
                            eng.reduce_max(
                                out=pool_max[m2][:, b : b + 1],
                                in_=psn2[:, None, :], axis=AX.X)
                    return f

                return ([t_stats, t_mean, t_rstd, t_bcast, t_norm]
                        + [t_n1(m) for m in range(FT)]
                        + [t_n2(m2) for m2 in range(HH // P)])

            hn1 = []
            for m in range(FT):
                hn1.append(pa.tile([P, TOK], BF16, name=f"hn1_{m}",
                                   tag=f"hn1_{m}"))
            pool_sum, pool_max = [], []
            for m2 in range(HH // P):
                pool_sum.append(pa.tile([P, BSH], BF16, name=f"pool_s{m2}",
                                        tag=f"pool_s{m2}"))
                pool_max.append(pa.tile([P, BSH], BF16, name=f"pool_m{m2}",
                                        tag=f"pool_m{m2}"))

            # ---------------- edge MLP over K x K pairs ----------------
            chunk_list = [(b, ib) for b in range(BSH) for ib in range(NCH)]
            h1ts = {}
            t_pre = {}

            def emit_h1_add(ci):
                b, ib = chunk_list[ci]
                t = cp.tile([P, FT, IBLK * K], BF16, name="t_pre", tag="t_pre")
                bsl = slice(b * K, (b + 1) * K)
                usl = slice(b * K + ib * IBLK, b * K + (ib + 1) * IBLK)
                for q in range(2):
                    msl = slice(2 * q, 2 * q + 2)
                    nc.gpsimd.tensor_tensor(
                        t[:, msl, :].rearrange("p m (i j) -> p m i j", i=IBLK),
                        V_sb[:, msl, None, bsl].broadcast_to(
                            [P, 2, IBLK, K]),
                        U_sb[:, msl, usl, None].broadcast_to(
                            [P, 2, IBLK, K]),
                        op=ALU.add)
                t_pre[ci] = t

            def emit_h1_cast(ci):
                t = t_pre.pop(ci)
                h1t = cp.tile([P, FT, IBLK * K], F8, name="h1t", tag="h1t")
                for q in range(2):
                    msl = slice(2 * q, 2 * q + 2)
                    nc.scalar.activation(h1t[:, msl, :], t[:, msl, :], AF.Relu)
                h1ts[ci] = h1t

            NTH = (IBLK * K) // MM_FREE

            slabs = {}
            thunk_q = []
            n1aw, n1ow_t, n1sw_t, n2w = [], [], [], []
            emit_h1_add(0)
            emit_h1_cast(0)
            if len(chunk_list) > 1:
                emit_h1_add(1)
                emit_h1_cast(1)
            if len(chunk_list) > 2:
                emit_h1_add(2)
            pending_red = None
            for ci, (b, ib) in enumerate(chunk_list):
                c0 = b * K + ib * IBLK
                h1t = h1ts.pop(ci)

                # e2: fp8 DR, h2 -> fp8
                h2t = cp.tile([P, FT, IBLK * K], F8, name="h2t", tag="h2t")
                ps2 = []
                for m in range(FT):
                    msl = slice(m * P, (m + 1) * P)
                    p2 = pp.tile([P, IBLK * K], F32, name=f"ps2_{m}", tag="ps")
                    for th in range(NTH):
                        tsl = slice(th * MM_FREE, (th + 1) * MM_FREE)
                        for si in range(2):
                            nc.tensor.matmul(
                                p2[:, tsl], e2q[si][:, :, msl],
                                h1t[:, 2 * si : 2 * si + 2, tsl],
                                start=(si == 0), stop=(si == 1),
                                perf_mode=DR)
                    ps2.append(p2)
                for m in range(FT):
                    if (m % 2 == 0) if N_H2_ACT == 2 else (m < N_H2_ACT):
                        nc.scalar.activation(h2t[:, m, :], ps2[m], AF.Relu,
                                             bias=bcol(4 + m))
                    else:
                        nc.vector.tensor_scalar(
                            h2t[:, m, :], ps2[m], bcol(4 + m), 0.0,
                            op0=ALU.add, op1=ALU.max)

                if ci + 3 < len(chunk_list):
                    emit_h1_add(ci + 3)
                if ci + 2 < len(chunk_list):
                    emit_h1_cast(ci + 2)

                # e3: fp8 DR; h3 evict + j-sum into agg_all
                h3t = cp.tile([P, FT, IBLK * K], BF16, name="h3t", tag="h3t")
                for m in range(FT):
                    msl = slice(m * P, (m + 1) * P)
                    p3 = pp.tile([P, IBLK * K], F32, name=f"ps3_{m}", tag="ps")
                    for th in range(NTH):
                        tsl = slice(th * MM_FREE, (th + 1) * MM_FREE)
                        for si in range(2):
                            nc.tensor.matmul(
                                p3[:, tsl], e3q[si][:, :, msl],
                                h2t[:, 2 * si : 2 * si + 2, tsl],
                                start=(si == 0), stop=(si == 1),
                                perf_mode=DR)
                    last_chunk = ci == len(chunk_list) - 1
                    if last_chunk or (
                            (m % 2 == 0) if N_H3_ACT == 2
                            else (m < N_H3_ACT)):
                        nc.scalar.activation(h3t[:, m, :], p3, AF.Relu,
                                             bias=bcol(8 + m))
                    else:
                        nc.vector.tensor_scalar(
                            h3t[:, m, :], p3, bcol(8 + m), 0.0,
                            op0=ALU.add, op1=ALU.max)
                # deferred one-instr reduce for previous chunk
                if pending_red is not None:
                    pr_h3, pr_c0 = pending_red
                    with nc.allow_low_precision(reason="bf16 agg"):
                        nc.vector.reduce_sum(
                            out=agg_all[:, :, pr_c0 : pr_c0 + IBLK],
                            in_=pr_h3[:].rearrange("p m (i j) -> p m i j",
                                                   i=IBLK),
                            axis=AX.X)
                pending_red = (h3t, c0)
                if ci == 1:
                    # node weights: DMA streams during early chunks
                    for k in range(FT):
                        n1aw.append(wload(f"n1aw{k}", n1aT_d, k, BF16))
                    n1ow_t.append(wload("n1ow", n1oT_d, dt=BF16))
                    n1sw_t.append(wload("n1sw", n1sT_d, dt=BF16))
                    for k in range(FT):
                        n2w.append(wload(f"n2w{k}", n2T_d, k, BF16))
                if LN_INLINE and ci > 0 and ci % NCH == 0:
                    thunk_q.extend(sample_thunks(ci // NCH - 1))
                for _ in range(THUNKS_PC):
                    if thunk_q:
                        thunk_q.pop(0)()
                if LN_INLINE and INC_STATS and ci >= (BSH - 1) * NCH + 1:
                    # incremental stats for the last sample's landed agg cols
                    if ci == (BSH - 1) * NCH + 1:
                        last_slab = ppl.tile([P, 512], F32, name="ln_slab",
                                             tag="ln_slab")
                        slabs["last"] = last_slab
                    pc0 = (BSH - 1) * K + (ci - 1 - (BSH - 1) * NCH) * IBLK
                    ln_stats(BSH - 1, slabs["last"],
                             slice(pc0, pc0 + IBLK), first=(pc0 % K == 0),
                             last=False)

            pr_h3, pr_c0 = pending_red
            with nc.allow_low_precision(reason="bf16 agg"):
                for m in range(FT):
                    nc.vector.reduce_sum(
                        out=agg_all[:, m, pr_c0 : pr_c0 + IBLK],
                        in_=pr_h3[:, m, :].rearrange("p (i j) -> p i j",
                                                     i=IBLK),
                        axis=AX.X)
            while thunk_q:
                thunk_q.pop(0)()
            if LN_INLINE:
                if INC_STATS:
                    ln_stats(BSH - 1, slabs["last"],
                             slice(pr_c0, pr_c0 + IBLK), first=False,
                             last=True)
                    ths = sample_thunks(BSH - 1, norm_eng=nc.vector,
                                        slab0=slabs["last"])
                else:
                    ths = sample_thunks(BSH - 1, norm_eng=nc.vector)
                # node_pre before the LN scalar chain to hide its latency
                for t in ths:
                    t()
            else:
                for b in range(BSH):
                    for t in sample_thunks(b):
                        t()

            # ---- head weights (loads emitted late; DMA overlaps edge) ----
            mu1w = [wload(f"mu1w{k}", mu1T_d, k, BF16) for k in range(2 * FT)]
            s1w = [wload(f"s1w{k}", s1T_d, k, BF16) for k in range(2 * FT)]
            mu2w = [wload(f"mu2w{k}", mu2T_d, k, BF16) for k in range(2)]
            s2w = [wload(f"s2w{k}", s2T_d, k, BF16) for k in range(2)]
            mu3w = wload("mu3w", mu3T_d, dt=BF16)
            s3w = wload("s3w", s3T_d, dt=BF16)

            xs = xst + pool_sum + pool_max

            hl1 = {"mu": [], "s": []}
            for tag, w1, bc1 in (("mu", mu1w, 20), ("s", s1w, 24)):
                for m in range(2):
                    msl = slice(m * P, (m + 1) * P)
                    ph = pp.tile([P, BSH], F32, name=f"p{tag}1_{m}", tag="ps")
                    for k2 in range(2 * FT):
                        nc.tensor.matmul(ph, w1[k2][:, msl], xs[k2],
                                         start=(k2 == 0),
                                         stop=(k2 == 2 * FT - 1))
                    hm = pa.tile([P, BSH], BF16, name=f"h{tag}1_{m}",
                                 tag=f"h{tag}1_{m}")
                    eng = nc.scalar if tag == "mu" else None
                    if eng is not None:
                        eng.activation(hm, ph, AF.Relu, bias=bcol(bc1 + m))
                    else:
                        nc.vector.tensor_scalar(hm, ph, bcol(bc1 + m), 0.0,
                                                op0=ALU.add, op1=ALU.max)
                    hl1[tag].append(hm)
            hm2 = {}
            for tag, w2, bc2 in (("mu", mu2w, 22), ("s", s2w, 26)):
                ph2 = pp.tile([P, BSH], F32, name=f"p{tag}2", tag="ps")
                for k2 in range(2):
                    nc.tensor.matmul(ph2, w2[k2], hl1[tag][k2],
                                     start=(k2 == 0), stop=(k2 == 1))
                h2t_ = pa.tile([P, BSH], BF16, name=f"h{tag}2", tag=f"h{tag}2")
                if tag == "mu":
                    nc.scalar.activation(h2t_, ph2, AF.Relu, bias=bcol(bc2))
                else:
                    nc.vector.tensor_scalar(h2t_, ph2, bcol(bc2), 0.0,
                                            op0=ALU.add, op1=ALU.max)
                hm2[tag] = h2t_
            ph3_mu = pp.tile([NA, BSH], F32, name="pmu3", tag="ps")
            nc.tensor.matmul(ph3_mu, mu3w, hm2["mu"], start=True, stop=True)
            ph3_s = pp.tile([NA, BSH], F32, name="ps3h", tag="ps")
            nc.tensor.matmul(ph3_s, s3w, hm2["s"], start=True, stop=True)

            mu_sb = pa.tile([NA, BSH], F32, name="mu_sb", tag="mu_sb")
            nc.scalar.activation(mu_sb, ph3_mu, AF.Identity,
                                 bias=bcol(23, rows=NA))
            nc.sync.dma_start(out=mu_d[:], in_=mu_sb)
            std_sb = pa.tile([NA, BSH], F32, name="std_sb", tag="std_sb")
            nc.vector.tensor_scalar(std_sb, ph3_s, bcol(27, rows=NA), 0.0,
                                    op0=ALU.add, op1=ALU.bypass)
            nc.sync.dma_start(out=std_d[:], in_=std_sb)

    _split_excess_waits(nc)
    return nc


def _q8(x):
    import ml_dtypes
    return np.asarray(x, np.float32).astype(ml_dtypes.float8_e4m3)


def _f8f(x):
    return _q8(x).astype(np.float32)


def prep_weights(inp):
    """Host-side weight prep -> dict of replicated arrays."""
    import ml_dtypes
    bf = ml_dtypes.bfloat16

    def fb(a):
        return np.ascontiguousarray(np.asarray(a, np.float32), dtype=bf)

    e1_w = np.asarray(inp["e1_w"], np.float32)
    e2_w = np.asarray(inp["e2_w"], np.float32)
    e3_w = np.asarray(inp["e3_w"], np.float32)
    n1_w = np.asarray(inp["n1_w"], np.float32)
    ln_g = np.asarray(inp["ln_g"], np.float32)
    ln_b = np.asarray(inp["ln_b"], np.float32)
    n2_b = np.asarray(inp["n2_b"], np.float32)
    mu1_w = np.asarray(inp["mu1_w"], np.float32)
    s1_w = np.asarray(inp["s1_w"], np.float32)
    b1 = np.asarray(inp["e1_b"], np.float32)
    b2 = np.asarray(inp["e2_b"], np.float32)
    b3 = np.asarray(inp["e3_b"], np.float32)

    d = {}
    A_ = e1_w[:, :D]
    C_ = e1_w[:, D:]
    d["e1aT"] = fb(A_.T)
    d["e1cT"] = fb(C_.T)

    def pack_dr(wt):
        out = {}
        for p in range(2):
            arr = np.zeros((P, 2, H), np.float32)
            for q_ in range(2):
                ks = (2 * p + q_) * P
                arr[:, q_, :] = wt[ks : ks + P, :]
            out[p] = _q8(arr)
        return out

    w2t = e2_w.T
    q2 = _f8f(w2t)
    for p, a in pack_dr(q2).items():
        d[f"e2q{p}"] = a
    e2_eff = q2.T

    w3t = e3_w.T
    q3 = _f8f(w3t)
    for p, a in pack_dr(q3).items():
        d[f"e3q{p}"] = a
    e3_eff = q3.T

    # ---- bias corrections via subsampled calibration means ----
    obs = np.asarray(inp["obs"], np.float32)
    o = obs.transpose(0, 2, 1).reshape(B * K, D)
    obf = np.asarray(np.asarray(o, np.float32).astype(bf), np.float32)
    Abf = np.asarray(fb(A_), np.float32)
    Cbf = np.asarray(fb(C_), np.float32)
    U = (obf @ Abf.T).reshape(B, K, H)
    V = (obf @ Cbf.T).reshape(B, K, H)
    jsub = np.arange(0, K, 8)
    m1 = np.zeros(H, np.float64)
    m2 = np.zeros(H, np.float64)
    for bi in range(B):
        h1 = np.maximum(U[bi][:, None, :] + V[bi][None, jsub, :] + b1, 0.0)
        h1 = _f8f(h1.reshape(-1, H))
        m1 += h1.mean(0) / B
        h2 = np.maximum(h1 @ e2_w.T + b2, 0.0)
        m2 += h2.mean(0) / B
    m1 = m1.astype(np.float32)
    m2 = m2.astype(np.float32)
    b2_eff = b2 - (e2_eff - e2_w) @ m1
    b3_eff = b3 - (e3_eff - e3_w) @ m2

    d["n1aT"] = fb((n1_w[:, D : D + H] * ln_g[None, :]).T.reshape(FT, P, H))
    d["n1oT"] = fb(n1_w[:, :D].T)
    d["n1sT"] = fb(n1_w[:, D + H :].T)
    d["n2T"] = fb(np.asarray(inp["n2_w"], np.float32).T.reshape(FT, P, HH))
    d["layerT"] = fb(np.asarray(inp["layer_w"], np.float32).T)

    mu1 = mu1_w.copy()
    mu1[:, H : H + HH] *= 1.0 / K
    d["mu1T"] = fb(mu1.T.reshape(2 * FT, P, 256))
    s1 = s1_w.copy()
    s1[:, H : H + HH] *= 1.0 / K
    d["s1T"] = fb(s1.T.reshape(2 * FT, P, 256))
    d["mu2T"] = fb(np.asarray(inp["mu2_w"], np.float32).T.reshape(2, P, 128))
    d["s2T"] = fb(np.asarray(inp["s2_w"], np.float32).T.reshape(2, P, 128))
    d["mu3T"] = fb(np.asarray(inp["mu3_w"], np.float32).T)
    d["s3T"] = fb(np.asarray(inp["s3_w"], np.float32).T)

    n1_b_eff = np.asarray(inp["n1_b"], np.float32) + n1_w[:, D : D + H] @ ln_b
    mu1_b_eff = (np.asarray(inp["mu1_b"], np.float32)
                 + (mu1_w[:, H : H + HH] + mu1_w[:, H + HH :]) @ n2_b)
    s1_b_eff = (np.asarray(inp["s1_b"], np.float32)
                + (s1_w[:, H : H + HH] + s1_w[:, H + HH :]) @ n2_b)

    bp = np.zeros((P, 32), np.float32)
    bp[:, 0:4] = b1.reshape(FT, P).T
    bp[:, 4:8] = b2_eff.reshape(FT, P).T
    bp[:, 8:12] = b3_eff.reshape(FT, P).T
    bp[:, 12:16] = n1_b_eff.reshape(FT, P).T
    bp[:, 16:20] = np.asarray(inp["layer_b"], np.float32).reshape(FT, P).T
    bp[:, 20:22] = mu1_b_eff.reshape(2, P).T
    bp[:, 22] = np.asarray(inp["mu2_b"], np.float32)
    bp[0:NA, 23] = np.asarray(inp["mu3_b"], np.float32)
    bp[:, 24:26] = s1_b_eff.reshape(2, P).T
    bp[:, 26] = np.asarray(inp["s2_b"], np.float32)
    bp[0:NA, 27] = np.asarray(inp["s3_b"], np.float32)
    d["bias_pack"] = bp
    return d


def make_in_maps(inputs):
    import ml_dtypes
    bf = ml_dtypes.bfloat16
    w = prep_weights(inputs)
    obs = np.asarray(inputs["obs"], np.float32)
    state = np.asarray(inputs["state"], np.float32)
    in_maps = []
    for c in range(NCORES):
        m = dict(w)
        ob = obs[c * BSH : (c + 1) * BSH]          # [BSH, D, K]
        m["obsT"] = np.ascontiguousarray(
            ob.transpose(1, 0, 2).reshape(D, TOK), dtype=bf)
        m["stateT"] = np.ascontiguousarray(
            state[c * BSH : (c + 1) * BSH].T, dtype=bf)
        in_maps.append(m)
    return in_maps


_NC_CACHE = {}


def get_nc():
    key = (WARMUP_MM, WARMUP2_MM, CHUNK_BUFS, N_H2_ACT, N_H3_ACT, MM_FREE,
           LN_INLINE, THUNKS_PC, INC_STATS)
    if key not in _NC_CACHE:
        _NC_CACHE[key] = build_bass()
    return _NC_CACHE[key]


def run(in_maps, trace=False, **kw):
    nc = get_nc()
    return run_bass_kernel_spmd(nc, in_maps, core_ids=list(range(NCORES)),
                                trace=trace, **kw)


def gather(res_list):
    mu = np.concatenate([r["mu"].T for r in res_list], axis=0)
    pre = np.concatenate([r["std"].T for r in res_list],
                         axis=0).astype(np.float64)
    std = np.clip(np.log1p(np.exp(pre)) + 0.001, 0.1, 2.0)
    return mu.astype(np.float32), std.astype(np.float32)


def kernel(**inputs):
    res = run(make_in_maps(inputs))
    return gather(res.results)


# revision 5
# speedup vs baseline: 1.0174x; 1.0072x over previous
"""Trainium2 Bass kernel (v21) for nn_ActorNetwork (GNN message passing actor).

Self-contained: hardcodes shapes B=32, K=64, D=4, DS=4, H=512, HH=256, NA=2.
Data-parallel over batch across 8 NeuronCores (4 samples/core).

Structure (per core, per 512-edge-token chunk):
- h1 = relu(U_i + V_j + b1): U=A@o, V=C@o+b1 once per core (PE); per chunk
  Pool broadcast-add (bf16) + ACT relu-cast to fp8.
- e2/e3: fp8 DoubleRow matmuls on PE (the bottleneck engine by design).
- h2/h3 PSUM evictions split ACT/DVE; j-reduction as one DVE instr per
  chunk into a bf16 agg_all tile.
- LN stats+normalize per sample, overlapped into the edge stream
  (normalize on Pool); node MLP + pools + heads in the tail with
  full-width instructions.
"""
import os as _os

import numpy as np

import concourse.bass as bass
import concourse.mybir as mybir
from concourse.bass_utils import run_bass_kernel_spmd
from concourse.tile import TileContext

# ---- problem constants ----
B, K, D, DS, H, HH, NA = 32, 64, 4, 4, 512, 256, 2
NCORES = 8
BSH = B // NCORES            # samples per core = 4
P = 128
FT = H // P                  # 4 feature tiles of hidden dim
TOK = BSH * K                # 256 node tokens per core
IBLK = 8                     # i-rows per edge chunk (8*64 = 512 tokens)
NCH = K // IBLK              # 8 chunks per sample
ET = K * K                   # 4096 edge tokens per sample

F32 = mybir.dt.float32
BF16 = mybir.dt.bfloat16
F8 = mybir.dt.float8e4
AF = mybir.ActivationFunctionType
AX = mybir.AxisListType
ALU = mybir.AluOpType
DR = mybir.MatmulPerfMode.DoubleRow

WARMUP_MM = int(_os.environ.get("K21_WARMUP_MM", "16"))
WARMUP2_MM = int(_os.environ.get("K21_WARMUP2_MM", "48"))
CHUNK_BUFS = int(_os.environ.get("K21_CHUNK_BUFS", "3"))
N_H2_ACT = int(_os.environ.get("K21_H2_ACT", "2"))   # h2 evicts on ACT (rest DVE)
N_H3_ACT = int(_os.environ.get("K21_H3_ACT", "2"))   # h3 evicts on ACT (rest DVE)
MM_FREE = int(_os.environ.get("K21_MM_FREE", "512"))  # moving cols per DR matmul
LN_INLINE = int(_os.environ.get("K21_LN_INLINE", "1"))  # per-sample LN in edge
THUNKS_PC = int(_os.environ.get("K21_THUNKS_PC", "2"))  # ln/node thunks per chunk
INC_STATS = int(_os.environ.get("K21_INC_STATS", "1"))  # incremental stats for last sample

EPS_S = (K * K) * 1e-5


def _split_excess_waits(nc, max_waits=1):
    """walrus in this container rejects >~2 sem waits on one instruction."""
    for f in nc.m.functions:
        for bb in f.blocks:
            insts = list(bb.instructions)
            new_list = []
            changed = False
            for inst in insts:
                si = inst.sync_info
                if si is not None and si.on_wait and len(si.on_wait) > max_waits:
                    waits = list(si.on_wait)
                    extra, keep = waits[:-max_waits], waits[-max_waits:]
                    for k0 in range(0, len(extra), max_waits):
                        chunk = extra[k0 : k0 + max_waits]
                        nop = mybir.InstNoOp(
                            name=f"{inst.name}-wsplit-{k0}",
                            engine=inst.engine,
                            ins=[],
                            outs=[],
                            sync_info=mybir.SyncInfo(on_wait=chunk, on_update=[]),
                        )
                        new_list.append(nop)
                        changed = True
                    si.on_wait = keep
                new_list.append(inst)
            if changed:
                bb.instructions = new_list


def build_bass():
    nc = bass.Bass("TRN2", debug=False, num_devices=NCORES)

    def dp(nm, sh, dt=F32):
        return nc.declare_dram_parameter(nm, sh, dt, isOutput=False)

    e1aT_d = dp("e1aT", [D, H], BF16)
    e1cT_d = dp("e1cT", [D, H], BF16)
    e2q_d = [dp(f"e2q{p}", [P, 2, H], F8) for p in range(2)]
    e3q_d = [dp(f"e3q{p}", [P, 2, H], F8) for p in range(2)]
    obs_d = dp("obsT", [D, TOK], BF16)
    st_d = dp("stateT", [DS, BSH], BF16)
    n1aT_d = dp("n1aT", [FT, P, H], BF16)
    n1oT_d = dp("n1oT", [D, H], BF16)
    n1sT_d = dp("n1sT", [DS, H], BF16)
    n2T_d = dp("n2T", [FT, P, HH], BF16)
    layerT_d = dp("layerT", [DS, H], BF16)
    mu1T_d = dp("mu1T", [2 * FT, P, 256], BF16)
    s1T_d = dp("s1T", [2 * FT, P, 256], BF16)
    mu2T_d = dp("mu2T", [2, P, 128], BF16)
    s2T_d = dp("s2T", [2, P, 128], BF16)
    mu3T_d = dp("mu3T", [P, NA], BF16)
    s3T_d = dp("s3T", [P, NA], BF16)
    bias_d = dp("bias_pack", [P, 32])
    mu_d = nc.declare_dram_parameter("mu", [NA, BSH], F32, isOutput=True)
    std_d = nc.declare_dram_parameter("std", [NA, BSH], F32, isOutput=True)

    with TileContext(nc) as tc:
        with (
            tc.tile_pool(name="w", bufs=1) as wp,
            tc.tile_pool(name="act", bufs=1) as pa,
            tc.tile_pool(name="chunk", bufs=CHUNK_BUFS) as cp,
            tc.tile_pool(name="ps", bufs=6, space="PSUM") as pp,
            tc.tile_pool(name="psln", bufs=2, space="PSUM") as ppl,
        ):
            def wload(nm, dram, idx=None, dt=F32):
                src = dram[:] if idx is None else dram[idx]
                t = wp.tile(list(src.shape), dt, name=nm, tag=nm)
                nc.sync.dma_start(out=t, in_=src)
                return t

            # ---- critical-path inputs first ----
            o_all = pa.tile([D, TOK], BF16, name="o_all", tag="o_all")
            nc.sync.dma_start(out=o_all, in_=obs_d[:])
            e1aT = wload("e1aT", e1aT_d, dt=BF16)
            e1cT = wload("e1cT", e1cT_d, dt=BF16)
            bias_t = wload("bias_t", bias_d)
            st_t = pa.tile([DS, BSH], BF16, name="st_t", tag="st_t")
            nc.sync.dma_start(out=st_t, in_=st_d[:])
            e2q = [wload(f"e2q{p}", e2q_d[p], dt=F8) for p in range(2)]
            e3q = [wload(f"e3q{p}", e3q_d[p], dt=F8) for p in range(2)]
            layerw = wload("layerw", layerT_d, dt=BF16)

            def bcol(i, rows=P):
                return bias_t[0:rows, i : i + 1]

            ones_col = pa.tile([P, 1], BF16, name="ones_col", tag="ones_col")
            nc.vector.memset(ones_col, 1.0)
            ones_row = pa.tile([1, P], F32, name="ones_row", tag="ones_row")
            nc.vector.memset(ones_row, 1.0)
            eps_t = pa.tile([1, 1], F32, name="eps_t", tag="eps_t")
            nc.vector.memset(eps_t, EPS_S)

            # trigger ACT table load early (overlaps DMA wait)
            dummy_a = pa.tile([1, 1], F32, name="dummy_a", tag="dummy_a")
            nc.scalar.activation(dummy_a, eps_t, AF.Relu)

            state_bc = pa.tile([DS, TOK], BF16, name="state_bc", tag="state_bc")
            nc.vector.tensor_copy(
                state_bc[:].rearrange("s (b k) -> s b k", b=BSH),
                st_t[:, :, None].broadcast_to([DS, BSH, K]),
            )

            # PE warmup while DMAs land (HAM un-throttle + clock ramp)
            wdu = pa.tile([P, 64], BF16, name="wdu", tag="wdu")
            nc.vector.memset(wdu, 0.0)
            psd = pp.tile([64, 64], F32, name="psd", tag="ps")
            for _w in range(WARMUP_MM):
                nc.tensor.matmul(psd, wdu, wdu, start=True, stop=True)

            # ---- U/V for e1-free h1: U = A@o, V = C@o + b1 ----
            U_sb = pa.tile([P, FT, TOK], BF16, name="U_sb", tag="U_sb")
            V_sb = pa.tile([P, FT, TOK], BF16, name="V_sb", tag="V_sb")
            for m in range(FT):
                msl = slice(m * P, (m + 1) * P)
                psu = pp.tile([P, TOK], F32, name=f"psu{m}", tag="ps")
                nc.tensor.matmul(psu, e1aT[:, msl], o_all, start=True,
                                 stop=True)
                nc.vector.tensor_copy(U_sb[:, m, :], psu)
                psv = pp.tile([P, TOK], F32, name=f"psv{m}", tag="ps")
                nc.tensor.matmul(psv, e1cT[:, msl], o_all, start=True,
                                 stop=True)
                nc.scalar.activation(V_sb[:, m, :], psv, AF.Identity,
                                     bias=bcol(0 + m))

            # st_feat early (no edge deps)
            xst = []
            for m in range(FT):
                msl = slice(m * P, (m + 1) * P)
                pst = pp.tile([P, BSH], F32, name=f"pst{m}", tag="ps")
                nc.tensor.matmul(pst, layerw[:, msl], st_t, start=True,
                                 stop=True)
                xm = pa.tile([P, BSH], BF16, name=f"xst{m}", tag=f"xst{m}")
                nc.scalar.activation(xm, pst, AF.Relu, bias=bcol(16 + m))
                xst.append(xm)


            psd2 = pp.tile([64, 64], F32, name="psd2", tag="ps")
            for _w in range(WARMUP2_MM):
                nc.tensor.matmul(psd2, wdu, wdu, start=True, stop=True)

            # agg/aggn accumulators [P, FT, TOK]
            agg_all = pa.tile([P, FT, TOK], BF16, name="agg_all", tag="agg_all")
            aggn_all = pa.tile([P, FT, TOK], BF16, name="aggn_all",
                               tag="aggn_all")

            # ---- per-sample LN stats + normalize ----
            def ln_stats(b, slab, cslice, first, last):
                # accumulate ones@agg and ones@agg^2 for agg cols cslice
                n = cslice.stop - cslice.start
                o0 = cslice.start - b * K
                sq = cp.tile([P, FT, n], BF16, name="sq_s", tag="sq_s")
                nc.scalar.activation(sq, agg_all[:, :, cslice], AF.Square)
                ps_sum = slab[0:1, o0 : o0 + n]
                ps_ssq = slab[0:1, K + o0 : K + o0 + n]
                for m in range(FT):
                    nc.tensor.matmul(ps_sum, ones_col, agg_all[:, m, cslice],
                                     start=(first and m == 0),
                                     stop=(last and m == FT - 1))
                for m in range(FT):
                    nc.tensor.matmul(ps_ssq, ones_col, sq[:, m, :],
                                     start=(first and m == 0),
                                     stop=(last and m == FT - 1))

            def sample_thunks(b, norm_eng=None, slab0=None):
                """LN + node MLP for sample b as a list of emission thunks.
                Each thunk's inputs are produced by earlier thunks/chunks so
                spreading them across chunk emissions avoids FIFO bubbles."""
                bsl = slice(b * K, (b + 1) * K)
                st = {}

                def t_stats():
                    if slab0 is not None:
                        st["slab"] = slab0
                    else:
                        st["slab"] = ppl.tile([P, 512], F32, name="ln_slab",
                                              tag="ln_slab")
                        ln_stats(b, st["slab"], bsl, True, True)

                def t_mean():
                    slab = st["slab"]
                    mean_r = cp.tile([1, K], F32, name="mean_r", tag="mean_r")
                    nc.vector.tensor_scalar_mul(mean_r, slab[0:1, 0:K],
                                                1.0 / H)
                    msq_r = cp.tile([1, K], F32, name="msq_r", tag="msq_r")
                    nc.vector.tensor_mul(msq_r, mean_r, mean_r)
                    var_r = cp.tile([1, K], F32, name="var_r", tag="var_r")
                    nc.vector.scalar_tensor_tensor(
                        var_r, slab[0:1, K : 2 * K], 1.0 / H, msq_r,
                        op0=ALU.mult, op1=ALU.subtract)
                    st["mean_r"], st["var_r"] = mean_r, var_r

                def t_rstd():
                    lnv_r = cp.tile([1, K], F32, name="lnv_r", tag="lnv_r")
                    nc.scalar.activation(lnv_r, st["var_r"], AF.Ln,
                                         bias=eps_t)
                    rstd_r = cp.tile([1, K], F32, name="rstd_r", tag="rstd_r")
                    nc.scalar.activation(rstd_r, lnv_r, AF.Exp, scale=-0.5)
                    st["rstd_r"] = rstd_r

                def t_bcast():
                    slab = st["slab"]
                    ps_mb = slab[:, 2 * K : 3 * K]
                    nc.tensor.matmul(ps_mb, ones_row, st["mean_r"],
                                     start=True, stop=True)
                    ps_rb = slab[:, 3 * K : 4 * K]
                    nc.tensor.matmul(ps_rb, ones_row, st["rstd_r"],
                                     start=True, stop=True)
                    mean_bc = cp.tile([P, K], F32, name="mean_bc",
                                      tag="mean_bc")
                    nc.scalar.copy(mean_bc, ps_mb)
                    rstd_bc = cp.tile([P, K], F32, name="rstd_bc",
                                      tag="rstd_bc")
                    nc.scalar.copy(rstd_bc, ps_rb)
                    st["mean_bc"], st["rstd_bc"] = mean_bc, rstd_bc

                def t_norm():
                    eng = norm_eng or nc.gpsimd
                    tmp = cp.tile([P, FT, K], BF16, name="ln_tmp",
                                  tag="ln_tmp")
                    eng.tensor_tensor(
                        tmp, agg_all[:, :, bsl],
                        st["mean_bc"][:, None, :].broadcast_to([P, FT, K]),
                        op=ALU.subtract)
                    eng.tensor_tensor(
                        aggn_all[:, :, bsl], tmp,
                        st["rstd_bc"][:, None, :].broadcast_to([P, FT, K]),
                        op=ALU.mult)

                def t_n1(m):
                    def f():
                        slab = st["slab"]
                        msl = slice(m * P, (m + 1) * P)
                        psn = slab[:, 256 + K * m : 256 + K * (m + 1)]
                        nc.tensor.matmul(psn, n1ow_t[0][:, msl],
                                         o_all[:, bsl],
                                         start=True, stop=False)
                        nc.tensor.matmul(psn, n1sw_t[0][:, msl],
                                         state_bc[:, bsl],
                                         start=False, stop=False)
                        for k2 in range(FT):
                            nc.tensor.matmul(psn, n1aw[k2][:, msl],
                                             aggn_all[:, k2, bsl],
                                             start=False,
                                             stop=(k2 == FT - 1))
                        nc.scalar.activation(hn1[m][:, bsl], psn, AF.Relu,
                                             bias=bcol(12 + m))
                    return f

                def t_n2(m2):
                    def f():
                        slab = st["slab"]
                        msl = slice(m2 * P, (m2 + 1) * P)
                        psn2 = slab[:, K * m2 : K * (m2 + 1)]
                        for k2 in range(FT):
                            nc.tensor.matmul(psn2, n2w[k2][:, msl],
                                             hn1[k2][:, bsl],
                                             start=(k2 == 0),
                                             stop=(k2 == FT - 1))
                        eng = nc.vector
                        with nc.allow_low_precision(reason="bf16 pool"):
                            eng.reduce_sum(
                                out=pool_sum[m2][:, b : b + 1],
                                in_=psn2[:, None, :], axis=AX.X)
                            eng.reduce_max(
                                out=pool_max[m2][:, b : b + 1],
                                in_=psn2[:, None, :], axis=AX.X)
                    return f

                return ([t_stats, t_mean, t_rstd, t_bcast, t_norm]
                        + [t_n1(m) for m in range(FT)]
                        + [t_n2(m2) for m2 in range(HH // P)])

            hn1 = []
            for m in range(FT):
                hn1.append(pa.tile([P, TOK], BF16, name=f"hn1_{m}",
                                   tag=f"hn1_{m}"))
            pool_sum, pool_max = [], []
            for m2 in range(HH // P):
                pool_sum.append(pa.tile([P, BSH], BF16, name=f"pool_s{m2}",
                                        tag=f"pool_s{m2}"))
                pool_max.append(pa.tile([P, BSH], BF16, name=f"pool_m{m2}",
                                        tag=f"pool_m{m2}"))

            # ---------------- edge MLP over K x K pairs ----------------
            chunk_list = [(b, ib) for b in range(BSH) for ib in range(NCH)]
            h1ts = {}
            t_pre = {}

            def emit_h1_add(ci):
                b, ib = chunk_list[ci]
                t = cp.tile([P, FT, IBLK * K], BF16, name="t_pre", tag="t_pre")
                bsl = slice(b * K, (b + 1) * K)
                usl = slice(b * K + ib * IBLK, b * K + (ib + 1) * IBLK)
                for q in range(2):
                    msl = slice(2 * q, 2 * q + 2)
                    nc.gpsimd.tensor_tensor(
                        t[:, msl, :].rearrange("p m (i j) -> p m i j", i=IBLK),
                        V_sb[:, msl, None, bsl].broadcast_to(
                            [P, 2, IBLK, K]),
                        U_sb[:, msl, usl, None].broadcast_to(
                            [P, 2, IBLK, K]),
                        op=ALU.add)
                t_pre[ci] = t

            def emit_h1_cast(ci):
                t = t_pre.pop(ci)
                h1t = cp.tile([P, FT, IBLK * K], F8, name="h1t", tag="h1t")
                for q in range(2):
                    msl = slice(2 * q, 2 * q + 2)
                    nc.scalar.activation(h1t[:, msl, :], t[:, msl, :], AF.Relu)
                h1ts[ci] = h1t

            NTH = (IBLK * K) // MM_FREE

            slabs = {}
            thunk_q = []
            n1aw, n1ow_t, n1sw_t, n2w = [], [], [], []
            emit_h1_add(0)
            emit_h1_cast(0)
            if len(chunk_list) > 1:
                emit_h1_add(1)
                emit_h1_cast(1)
            if len(chunk_list) > 2:
                emit_h1_add(2)
            pending_red = None
            for ci, (b, ib) in enumerate(chunk_list):
                c0 = b * K + ib * IBLK
                h1t = h1ts.pop(ci)

                # e2: fp8 DR, h2 -> fp8
                h2t = cp.tile([P, FT, IBLK * K], F8, name="h2t", tag="h2t")
                ps2 = []
                for m in range(FT):
                    msl = slice(m * P, (m + 1) * P)
                    p2 = pp.tile([P, IBLK * K], F32, name=f"ps2_{m}", tag="ps")
                    for th in range(NTH):
                        tsl = slice(th * MM_FREE, (th + 1) * MM_FREE)
                        for si in range(2):
                            nc.tensor.matmul(
                                p2[:, tsl], e2q[si][:, :, msl],
                                h1t[:, 2 * si : 2 * si + 2, tsl],
                                start=(si == 0), stop=(si == 1),
                                perf_mode=DR)
                    ps2.append(p2)
                for m in range(FT):
                    if (m % 2 == 0) if N_H2_ACT == 2 else (m < N_H2_ACT):
                        nc.scalar.activation(h2t[:, m, :], ps2[m], AF.Relu,
                                             bias=bcol(4 + m))
                    else:
                        nc.vector.tensor_scalar(
                            h2t[:, m, :], ps2[m], bcol(4 + m), 0.0,
                            op0=ALU.add, op1=ALU.max)

                if ci + 3 < len(chunk_list):
                    emit_h1_add(ci + 3)
                if ci + 2 < len(chunk_list):
                    emit_h1_cast(ci + 2)

                # e3: fp8 DR; h3 evict + j-sum into agg_all
                h3t = cp.tile([P, FT, IBLK * K], BF16, name="h3t", tag="h3t")
                for m in range(FT):
                    msl = slice(m * P, (m + 1) * P)
                    p3 = pp.tile([P, IBLK * K], F32, name=f"ps3_{m}", tag="ps")
                    for th in range(NTH):
                        tsl = slice(th * MM_FREE, (th + 1) * MM_FREE)
                        for si in range(2):
                            nc.tensor.matmul(
                                p3[:, tsl], e3q[si][:, :, msl],
                                h2t[:, 2 * si : 2 * si + 2, tsl],
                                start=(si == 0), stop=(si == 1),
                                perf_mode=DR)
                    last_chunk = ci >= len(chunk_list) - 2
                    if last_chunk or (
                            (m % 2 == 0) if N_H3_ACT == 2
                            else (m < N_H3_ACT)):
                        nc.scalar.activation(h3t[:, m, :], p3, AF.Relu,
                                             bias=bcol(8 + m))
                    else:
                        nc.vector.tensor_scalar(
                            h3t[:, m, :], p3, bcol(8 + m), 0.0,
                            op0=ALU.add, op1=ALU.max)
                # deferred one-instr reduce for previous chunk
                if pending_red is not None:
                    pr_h3, pr_c0 = pending_red
                    with nc.allow_low_precision(reason="bf16 agg"):
                        nc.vector.reduce_sum(
                            out=agg_all[:, :, pr_c0 : pr_c0 + IBLK],
                            in_=pr_h3[:].rearrange("p m (i j) -> p m i j",
                                                   i=IBLK),
                            axis=AX.X)
                pending_red = (h3t, c0)
                if ci == 1:
                    # node weights: DMA streams during early chunks
                    for k in range(FT):
                        n1aw.append(wload(f"n1aw{k}", n1aT_d, k, BF16))
                    n1ow_t.append(wload("n1ow", n1oT_d, dt=BF16))
                    n1sw_t.append(wload("n1sw", n1sT_d, dt=BF16))
                    for k in range(FT):
                        n2w.append(wload(f"n2w{k}", n2T_d, k, BF16))
                if LN_INLINE and ci > 0 and ci % NCH == 0:
                    thunk_q.extend(sample_thunks(ci // NCH - 1))
                for _ in range(THUNKS_PC):
                    if thunk_q:
                        thunk_q.pop(0)()
                if LN_INLINE and INC_STATS and ci >= (BSH - 1) * NCH + 1:
                    # incremental stats for the last sample's landed agg cols
                    if ci == (BSH - 1) * NCH + 1:
                        last_slab = ppl.tile([P, 512], F32, name="ln_slab",
                                             tag="ln_slab")
                        slabs["last"] = last_slab
                    pc0 = (BSH - 1) * K + (ci - 1 - (BSH - 1) * NCH) * IBLK
                    ln_stats(BSH - 1, slabs["last"],
                             slice(pc0, pc0 + IBLK), first=(pc0 % K == 0),
                             last=False)

            pr_h3, pr_c0 = pending_red
            with nc.allow_low_precision(reason="bf16 agg"):
                for m in range(FT):
                    nc.vector.reduce_sum(
                        out=agg_all[:, m, pr_c0 : pr_c0 + IBLK],
                        in_=pr_h3[:, m, :].rearrange("p (i j) -> p i j",
                                                     i=IBLK),
                        axis=AX.X)
            while thunk_q:
                thunk_q.pop(0)()
            if LN_INLINE:
                if INC_STATS:
                    ln_stats(BSH - 1, slabs["last"],
                             slice(pr_c0, pr_c0 + IBLK), first=False,
                             last=True)
                    ths = sample_thunks(BSH - 1, norm_eng=nc.vector,
                                        slab0=slabs["last"])
                else:
                    ths = sample_thunks(BSH - 1, norm_eng=nc.vector)
                # node_pre before the LN scalar chain to hide its latency
                for t in ths:
                    t()
            else:
                for b in range(BSH):
                    for t in sample_thunks(b):
                        t()

            # ---- head weights (loads emitted late; DMA overlaps edge) ----
            mu1w = [wload(f"mu1w{k}", mu1T_d, k, BF16) for k in range(2 * FT)]
            s1w = [wload(f"s1w{k}", s1T_d, k, BF16) for k in range(2 * FT)]
            mu2w = [wload(f"mu2w{k}", mu2T_d, k, BF16) for k in range(2)]
            s2w = [wload(f"s2w{k}", s2T_d, k, BF16) for k in range(2)]
            mu3w = wload("mu3w", mu3T_d, dt=BF16)
            s3w = wload("s3w", s3T_d, dt=BF16)

            xs = xst + pool_sum + pool_max

            hl1 = {"mu": [], "s": []}
            for tag, w1, bc1 in (("mu", mu1w, 20), ("s", s1w, 24)):
                for m in range(2):
                    msl = slice(m * P, (m + 1) * P)
                    ph = pp.tile([P, BSH], F32, name=f"p{tag}1_{m}", tag="ps")
                    for k2 in range(2 * FT):
                        nc.tensor.matmul(ph, w1[k2][:, msl], xs[k2],
                                         start=(k2 == 0),
                                         stop=(k2 == 2 * FT - 1))
                    hm = pa.tile([P, BSH], BF16, name=f"h{tag}1_{m}",
                                 tag=f"h{tag}1_{m}")
                    eng = nc.scalar if tag == "mu" else None
                    if eng is not None:
                        eng.activation(hm, ph, AF.Relu, bias=bcol(bc1 + m))
                    else:
                        nc.vector.tensor_scalar(hm, ph, bcol(bc1 + m), 0.0,
                                                op0=ALU.add, op1=ALU.max)
                    hl1[tag].append(hm)
            hm2 = {}
            for tag, w2, bc2 in (("mu", mu2w, 22), ("s", s2w, 26)):
                ph2 = pp.tile([P, BSH], F32, name=f"p{tag}2", tag="ps")
                for k2 in range(2):
                    nc.tensor.matmul(ph2, w2[k2], hl1[tag][k2],
                                     start=(k2 == 0), stop=(k2 == 1))
                h2t_ = pa.tile([P, BSH], BF16, name=f"h{tag}2", tag=f"h{tag}2")
                if tag == "mu":
                    nc.scalar.activation(h2t_, ph2, AF.Relu, bias=bcol(bc2))
                else:
                    nc.vector.tensor_scalar(h2t_, ph2, bcol(bc2), 0.0,
                                            op0=ALU.add, op1=ALU.max)
                hm2[tag] = h2t_
            ph3_mu = pp.tile([NA, BSH], F32, name="pmu3", tag="ps")
            nc.tensor.matmul(ph3_mu, mu3w, hm2["mu"], start=True, stop=True)
            ph3_s = pp.tile([NA, BSH], F32, name="ps3h", tag="ps")
            nc.tensor.matmul(ph3_s, s3w, hm2["s"], start=True, stop=True)

            mu_sb = pa.tile([NA, BSH], F32, name="mu_sb", tag="mu_sb")
            nc.scalar.activation(mu_sb, ph3_mu, AF.Identity,
                                 bias=bcol(23, rows=NA))
            nc.sync.dma_start(out=mu_d[:], in_=mu_sb)
            std_sb = pa.tile([NA, BSH], F32, name="std_sb", tag="std_sb")
            nc.vector.tensor_scalar(std_sb, ph3_s, bcol(27, rows=NA), 0.0,
                                    op0=ALU.add, op1=ALU.bypass)
            nc.sync.dma_start(out=std_d[:], in_=std_sb)

    _split_excess_waits(nc)
    return nc


def _q8(x):
    import ml_dtypes
    return np.asarray(x, np.float32).astype(ml_dtypes.float8_e4m3)


def _f8f(x):
    return _q8(x).astype(np.float32)


def prep_weights(inp):
    """Host-side weight prep -> dict of replicated arrays."""
    import ml_dtypes
    bf = ml_dtypes.bfloat16

    def fb(a):
        return np.ascontiguousarray(np.asarray(a, np.float32), dtype=bf)

    e1_w = np.asarray(inp["e1_w"], np.float32)
    e2_w = np.asarray(inp["e2_w"], np.float32)
    e3_w = np.asarray(inp["e3_w"], np.float32)
    n1_w = np.asarray(inp["n1_w"], np.float32)
    ln_g = np.asarray(inp["ln_g"], np.float32)
    ln_b = np.asarray(inp["ln_b"], np.float32)
    n2_b = np.asarray(inp["n2_b"], np.float32)
    mu1_w = np.asarray(inp["mu1_w"], np.float32)
    s1_w = np.asarray(inp["s1_w"], np.float32)
    b1 = np.asarray(inp["e1_b"], np.float32)
    b2 = np.asarray(inp["e2_b"], np.float32)
    b3 = np.asarray(inp["e3_b"], np.float32)

    d = {}
    A_ = e1_w[:, :D]
    C_ = e1_w[:, D:]
    d["e1aT"] = fb(A_.T)
    d["e1cT"] = fb(C_.T)

    def pack_dr(wt):
        out = {}
        for p in range(2):
            arr = np.zeros((P, 2, H), np.float32)
            for q_ in range(2):
                ks = (2 * p + q_) * P
                arr[:, q_, :] = wt[ks : ks + P, :]
            out[p] = _q8(arr)
        return out

    w2t = e2_w.T
    q2 = _f8f(w2t)
    for p, a in pack_dr(q2).items():
        d[f"e2q{p}"] = a
    e2_eff = q2.T

    w3t = e3_w.T
    q3 = _f8f(w3t)
    for p, a in pack_dr(q3).items():
        d[f"e3q{p}"] = a
    e3_eff = q3.T

    # ---- bias corrections via subsampled calibration means ----
    obs = np.asarray(inp["obs"], np.float32)
    o = obs.transpose(0, 2, 1).reshape(B * K, D)
    obf = np.asarray(np.asarray(o, np.float32).astype(bf), np.float32)
    Abf = np.asarray(fb(A_), np.float32)
    Cbf = np.asarray(fb(C_), np.float32)
    U = (obf @ Abf.T).reshape(B, K, H)
    V = (obf @ Cbf.T).reshape(B, K, H)
    jsub = np.arange(0, K, 8)
    m1 = np.zeros(H, np.float64)
    m2 = np.zeros(H, np.float64)
    for bi in range(B):
        h1 = np.maximum(U[bi][:, None, :] + V[bi][None, jsub, :] + b1, 0.0)
        h1 = _f8f(h1.reshape(-1, H))
        m1 += h1.mean(0) / B
        h2 = np.maximum(h1 @ e2_w.T + b2, 0.0)
        m2 += h2.mean(0) / B
    m1 = m1.astype(np.float32)
    m2 = m2.astype(np.float32)
    b2_eff = b2 - (e2_eff - e2_w) @ m1
    b3_eff = b3 - (e3_eff - e3_w) @ m2

    d["n1aT"] = fb((n1_w[:, D : D + H] * ln_g[None, :]).T.reshape(FT, P, H))
    d["n1oT"] = fb(n1_w[:, :D].T)
    d["n1sT"] = fb(n1_w[:, D + H :].T)
    d["n2T"] = fb(np.asarray(inp["n2_w"], np.float32).T.reshape(FT, P, HH))
    d["layerT"] = fb(np.asarray(inp["layer_w"], np.float32).T)

    mu1 = mu1_w.copy()
    mu1[:, H : H + HH] *= 1.0 / K
    d["mu1T"] = fb(mu1.T.reshape(2 * FT, P, 256))
    s1 = s1_w.copy()
    s1[:, H : H + HH] *= 1.0 / K
    d["s1T"] = fb(s1.T.reshape(2 * FT, P, 256))
    d["mu2T"] = fb(np.asarray(inp["mu2_w"], np.float32).T.reshape(2, P, 128))
    d["s2T"] = fb(np.asarray(inp["s2_w"], np.float32).T.reshape(2, P, 128))
    d["mu3T"] = fb(np.asarray(inp["mu3_w"], np.float32).T)
    d["s3T"] = fb(np.asarray(inp["s3_w"], np.float32).T)

    n1_b_eff = np.asarray(inp["n1_b"], np.float32) + n1_w[:, D : D + H] @ ln_b
    mu1_b_eff = (np.asarray(inp["mu1_b"], np.float32)
                 + (mu1_w[:, H : H + HH] + mu1_w[:, H + HH :]) @ n2_b)
    s1_b_eff = (np.asarray(inp["s1_b"], np.float32)
                + (s1_w[:, H : H + HH] + s1_w[:, H + HH :]) @ n2_b)

    bp = np.zeros((P, 32), np.float32)
    bp[:, 0:4] = b1.reshape(FT, P).T
    bp[:, 4:8] = b2_eff.reshape(FT, P).T
    bp[:, 8:12] = b3_eff.reshape(FT, P).T
    bp[:, 12:16] = n1_b_eff.reshape(FT, P).T
    bp[:, 16:20] = np.asarray(inp["layer_b"], np.float32).reshape(FT, P).T
    bp[:, 20:22] = mu1_b_eff.reshape(2, P).T
    bp[:, 22] = np.asarray(inp["mu2_b"], np.float32)
    bp[0:NA, 23] = np.asarray(inp["mu3_b"], np.float32)
    bp[:, 24:26] = s1_b_eff.reshape(2, P).T
    bp[:, 26] = np.asarray(inp["s2_b"], np.float32)
    bp[0:NA, 27] = np.asarray(inp["s3_b"], np.float32)
    d["bias_pack"] = bp
    return d


def make_in_maps(inputs):
    import ml_dtypes
    bf = ml_dtypes.bfloat16
    w = prep_weights(inputs)
    obs = np.asarray(inputs["obs"], np.float32)
    state = np.asarray(inputs["state"], np.float32)
    in_maps = []
    for c in range(NCORES):
        m = dict(w)
        ob = obs[c * BSH : (c + 1) * BSH]          # [BSH, D, K]
        m["obsT"] = np.ascontiguousarray(
            ob.transpose(1, 0, 2).reshape(D, TOK), dtype=bf)
        m["stateT"] = np.ascontiguousarray(
            state[c * BSH : (c + 1) * BSH].T, dtype=bf)
        in_maps.append(m)
    return in_maps


_NC_CACHE = {}


def get_nc():
    key = (WARMUP_MM, WARMUP2_MM, CHUNK_BUFS, N_H2_ACT, N_H3_ACT, MM_FREE,
           LN_INLINE, THUNKS_PC, INC_STATS)
    if key not in _NC_CACHE:
        _NC_CACHE[key] = build_bass()
    return _NC_CACHE[key]


def run(in_maps, trace=False, **kw):
    nc = get_nc()
    return run_bass_kernel_spmd(nc, in_maps, core_ids=list(range(NCORES)),
                                trace=trace, **kw)


def gather(res_list):
    mu = np.concatenate([r["mu"].T for r in res_list], axis=0)
    pre = np.concatenate([r["std"].T for r in res_list],
                         axis=0).astype(np.float64)
    std = np.clip(np.log1p(np.exp(pre)) + 0.001, 0.1, 2.0)
    return mu.astype(np.float32), std.astype(np.float32)


def kernel(**inputs):
    res = run(make_in_maps(inputs))
    return gather(res.results)
